# revision 19
# baseline (speedup 1.0000x reference)
"""GATv2FeatureExtractor Trainium2 kernel (8 NeuronCores, edge-parallel by dst).

Strategy
--------
Edges are sorted by destination node and sharded into 8 contiguous dst ranges
(6250 nodes per core).  Each core processes its edges in fixed node-windows
(55 nodes for GAT layer 1, 119 for layer 2); the weighted message scatter is a
one-hot matmul accumulating into a PSUM window, so no cross-core reduction is
needed.  Only one collective runs: an AllGather of the layer-2 source table
(h1 @ Wl2) between the layers.

Per-core node rotation keeps the SPMD program identical on all cores: every
core's own nodes occupy rows 0:6250 of its tables, and gather indices are
remapped on the host.

GATv2 math is restructured so no per-edge softmax max pass is needed
(alpha is O(1) for this model; exp() cannot overflow), and
att . LeakyReLU(s) = 0.6*(att . s) + 0.4*sum_c |att_c * s_c|
where the linear term rides as extra matmul columns and the abs term uses the
scalar engine's Abs+accumulate.  All biases are folded into matmul ones-rows.
"""

import os
import sys

import numpy as np

if os.path.isdir("/opt/trn_rl_repo") and "/opt/trn_rl_repo" not in sys.path:
    sys.path.insert(0, "/opt/trn_rl_repo")

import concourse.bacc as bacc
import concourse.bass as bass
import concourse.mybir as mybir
import concourse.tile as tile
from concourse.bass import IndirectOffsetOnAxis
from concourse.bass_utils import run_bass_kernel_spmd

F32 = mybir.dt.float32
I32 = mybir.dt.int32
AF = mybir.ActivationFunctionType
ALU = mybir.AluOpType

NCORES = 8
P = 128
NEG = 0.2  # leaky relu slope

# model dims (hardcoded per spec)
F_IN, ED, HID, H, OUT = 32, 8, 64, 4, 256
HC = H * HID  # 256


def _ceil_div(a, b):
    return -(-a // b)


# ----------------------------------------------------------------------------
# host-side preprocessing
# ----------------------------------------------------------------------------

def _window_plan(dst_sorted, n, npc, span):
    """Edge ranges for fixed `span`-node windows on each core.

    Returns (nw, edge ranges [ncore][nw] as (e0, e1), max chunk count)."""
    nw = _ceil_div(npc, span)
    ranges = []
    kmax = 1
    for c in range(NCORES):
        lo = c * npc
        rows = []
        for w in range(nw):
            a = lo + w * span
            b = min(lo + (w + 1) * span, lo + npc)
            e0 = np.searchsorted(dst_sorted, a, side="left")
            e1 = np.searchsorted(dst_sorted, b, side="left")
            rows.append((int(e0), int(e1)))
            kmax = max(kmax, _ceil_div(max(e1 - e0, 1), P))
        ranges.append(rows)
    return nw, ranges, kmax


def _pack_edges(src_s, dst_s, ea_s, ranges, nw, k, npc, span, n, rotate):
    """Build per-core [128, nw*k] int32 src / f32 dstlocal / [128, nw*k*8] ea."""
    out = []
    for c in range(NCORES):
        lo = c * npc
        nchunk = nw * k
        src_f = np.zeros(nchunk * P, np.int32)
        dst_f = np.full(nchunk * P, -1.0, np.float32)
        ea_f = np.zeros((nchunk * P, ED), np.float32)
        for w in range(nw):
            e0, e1 = ranges[c][w]
            m = e1 - e0
            if m == 0:
                continue
            pos = w * k * P + np.arange(m)
            s = src_s[e0:e1]
            if rotate:
                s = (s - lo) % n
            src_f[pos] = s
            dst_f[pos] = (dst_s[e0:e1] - lo - w * span).astype(np.float32)
            ea_f[pos] = ea_s[e0:e1]
        src_a = src_f.reshape(nchunk, P).T.copy()  # [128, nchunk]
        dst_a = dst_f.reshape(nchunk, P).T.copy()
        ea_a = ea_f.reshape(nchunk, P, ED).transpose(1, 0, 2).reshape(P, nchunk * ED).copy()
        out.append((src_a, dst_a, ea_a))
    return out


def _prep_host(inputs, n, e, npc, w1, w2):
    """All host-side numpy preprocessing. Returns (meta, per-core in_maps)."""
    x = np.asarray(inputs["x"], np.float32)
    ei = np.asarray(inputs["edge_index"])
    ea = np.asarray(inputs["edge_attr"], np.float32)
    src = ei[0].astype(np.int64)
    dst = ei[1].astype(np.int64)

    order = np.argsort(dst, kind="stable")
    src_s = src[order].astype(np.int32)
    dst_s = dst[order].astype(np.int32)
    ea_s = ea[order]

    nw1, r1, k1 = _window_plan(dst_s, n, npc, w1)
    nw2, r2, k2 = _window_plan(dst_s, n, npc, w2)

    packed1 = _pack_edges(src_s, dst_s, ea_s, r1, nw1, k1, npc, w1, n, rotate=True)
    packed2 = _pack_edges(src_s, dst_s, ea_s, r2, nw2, k2, npc, w2, n, rotate=False)

    # --- weights ---
    W1 = np.asarray(inputs["W1"], np.float32); b1 = np.asarray(inputs["b1"], np.float32)
    W2 = np.asarray(inputs["W2"], np.float32); b2 = np.asarray(inputs["b2"], np.float32)
    Wl1 = np.asarray(inputs["Wl1"], np.float32); bl1 = np.asarray(inputs["bl1"], np.float32)
    Wr1 = np.asarray(inputs["Wr1"], np.float32); br1 = np.asarray(inputs["br1"], np.float32)
    We1 = np.asarray(inputs["We1"], np.float32)
    att1 = np.asarray(inputs["att1"], np.float32)  # [4, 64]
    bias1 = np.asarray(inputs["bias1"], np.float32)
    Wl2 = np.asarray(inputs["Wl2"], np.float32); bl2 = np.asarray(inputs["bl2"], np.float32)
    Wr2 = np.asarray(inputs["Wr2"], np.float32); br2 = np.asarray(inputs["br2"], np.float32)
    We2 = np.asarray(inputs["We2"], np.float32)
    att2 = np.asarray(inputs["att2"], np.float32)  # [1, 256]
    bias2 = np.asarray(inputs["bias2"], np.float32)

    consts = {}
    consts["ident"] = np.eye(P, dtype=np.float32)
    consts["iota"] = np.tile(np.arange(P, dtype=np.float32), (P, 1))
    consts["ones1"] = np.ones((1, P), np.float32)
    consts["mlp1"] = np.concatenate([W1, b1[None, :]], 0)  # [33, 64]
    consts["mlp2"] = np.concatenate([W2, b2[None, :]], 0)  # [65, 64]

    # layer-1 stacked rhs: |att|-scaled features permuted per head by sign(att),
    # plus 0.6 * linear attention columns.
    # att.LReLU(s) = 0.6*att.s + 0.4*(sum_pos |att*s| - sum_neg |att*s|)
    arow1 = att1.reshape(HC)
    abs1 = np.abs(arow1)
    amat1 = np.zeros((HC, H), np.float32)
    for h in range(H):
        amat1[h * HID:(h + 1) * HID, h] = att1[h]
    perm1 = np.concatenate([
        h * HID + np.concatenate([np.where(att1[h] > 0)[0], np.where(att1[h] <= 0)[0]])
        for h in range(H)]).astype(np.int64)
    npos1 = [int((att1[h] > 0).sum()) for h in range(H)]
    brow1 = (bl1 + br1)[None, :]  # [1, 256]

    def _aug1(m):  # [k, 256] -> [k, 260]
        return np.concatenate([(m * abs1[None, :])[:, perm1], 0.6 * (m @ amat1)], 1)

    consts["rhs1c"] = np.concatenate([_aug1(Wl1), _aug1(We1), _aug1(brow1)], 0)  # [73, 260]
    consts["wr1"] = _aug1(Wr1)  # [64, 260]
    rx = np.zeros((P, HC), np.float32)
    rx[0:HID] = Wl1
    rx[72] = bl1 + bias1
    consts["rhsxl1"] = rx  # [128, 256] payload rhs (unscaled + folded biases)

    # layer-2: columns stay unscaled/unpermuted; the signed-att weighting of the
    # abs term is applied per-edge via scalar_tensor_tensor against att2b.
    arow2 = att2.reshape(HC)

    def _aug2(m):  # [k, 256] -> [k, 257]
        return np.concatenate([m, 0.6 * (m @ arow2[:, None])], 1)

    brow2 = (br2 - bias2)[None, :]
    consts["rhs2c"] = np.concatenate([_aug2(We2), _aug2(brow2)], 0)  # [9, 257]
    wr2 = _aug2(Wr2)  # [256, 257]
    consts["wr2a"], consts["wr2b"] = wr2[0:P].copy(), wr2[P:2 * P].copy()
    wl2aug = np.concatenate([Wl2, 0.6 * (Wl2 @ arow2[:, None])], 1)  # [256, 257]
    consts["wl2a"], consts["wl2b"] = wl2aug[0:P].copy(), wl2aug[P:2 * P].copy()
    xb = (bl2 + bias2)[None, :]
    consts["xl2bias"] = np.concatenate([xb, 0.6 * (xb @ arow2[:, None])], 1)  # [1, 257]
    consts["att2b"] = np.tile(0.4 * arow2[None, :], (P, 1))  # [128, 256]

    # MLP input, transposed with ones row, per-core rotated, padded to 512 cols
    nch0 = _ceil_div(n, 512)
    npad = nch0 * 512
    xt_base = np.concatenate([x.T, np.ones((1, n), np.float32)], 0)  # [33, n]

    in_maps = []
    for c in range(NCORES):
        lo = c * npc
        rot = np.concatenate([np.arange(lo, n), np.arange(0, lo)])
        xt = np.zeros((F_IN + 1, npad), np.float32)
        xt[:, :n] = xt_base[:, rot]
        m = dict(consts)
        m["xt"] = xt
        m["src1"], m["dstl1"], m["ea1"] = packed1[c]
        m["src2"], m["dstl2"], m["ea2"] = packed2[c]
        in_maps.append(m)

    meta = dict(n=n, npc=npc, npad=npad, nch0=nch0,
                w1=w1, nw1=nw1, k1=int(k1), w2=w2, nw2=nw2, k2=int(k2),
                npos1=npos1)
    return meta, in_maps


# ----------------------------------------------------------------------------
# device program
# ----------------------------------------------------------------------------

def _build_nc(meta, debug=False):
    n, npc, npad, nch0 = meta["n"], meta["npc"], meta["npad"], meta["nch0"]
    w1, nw1, k1 = meta["w1"], meta["nw1"], meta["k1"]
    w2, nw2, k2 = meta["w2"], meta["nw2"], meta["k2"]
    npos1 = meta["npos1"]

    nc = bacc.Bacc("TRN2", target_bir_lowering=False, num_devices=NCORES)

    def din(name, shape, dtype=F32):
        return nc.dram_tensor(name, shape, dtype, kind="ExternalInput")

    ident_d = din("ident", [P, P])
    iota_d = din("iota", [P, P])
    ones1_d = din("ones1", [1, P])
    mlp1_d = din("mlp1", [F_IN + 1, HID])
    mlp2_d = din("mlp2", [HID + 1, HID])
    rhs1c_d = din("rhs1c", [73, HC + H])
    wr1_d = din("wr1", [HID, HC + H])
    rhsxl1_d = din("rhsxl1", [P, HC])
    rhs2c_d = din("rhs2c", [9, HC + 1])
    wr2a_d = din("wr2a", [P, HC + 1]); wr2b_d = din("wr2b", [P, HC + 1])
    wl2a_d = din("wl2a", [P, HC + 1]); wl2b_d = din("wl2b", [P, HC + 1])
    xl2bias_d = din("xl2bias", [1, HC + 1])
    att2b_d = din("att2b", [P, HC])
    xt_d = din("xt", [F_IN + 1, npad])
    src1_d = din("src1", [P, nw1 * k1], I32)
    dstl1_d = din("dstl1", [P, nw1 * k1])
    ea1_d = din("ea1", [P, nw1 * k1 * ED])
    src2_d = din("src2", [P, nw2 * k2], I32)
    dstl2_d = din("dstl2", [P, nw2 * k2])
    ea2_d = din("ea2", [P, nw2 * k2 * ED])
    out_d = nc.dram_tensor("out", [npc, HC], F32, kind="ExternalOutput")
    if debug:
        dbg_h = nc.dram_tensor("dbg_h", [npad, HID], F32, kind="ExternalOutput")
        dbg_h1 = nc.dram_tensor("dbg_h1", [npc, HC], F32, kind="ExternalOutput")
        dbg_xf = nc.dram_tensor("dbg_xf", [NCORES * npc, HC + 1], F32, kind="ExternalOutput")

    with tile.TileContext(nc) as tc:
        with (
            tc.tile_pool(name="dram", bufs=1, space="DRAM") as dram,
            tc.tile_pool(name="const", bufs=1) as cpool,
            tc.tile_pool(name="win", bufs=2) as wpool,
            tc.tile_pool(name="chunk", bufs=3) as kpool,
            tc.tile_pool(name="ps2", bufs=2, space="PSUM") as ps2,
            tc.tile_pool(name="ps1", bufs=1, space="PSUM") as ps1,
        ):
            h_full = dram.tile([npad, HID], F32)
            h1loc = dram.tile([npc, HC], F32)
            xl2loc = dram.tile([npc, HC + 1], F32)
            xl2full = dram.tile([NCORES * npc, HC + 1], F32)

            # persistent consts in SBUF
            ident = cpool.tile([P, P], F32); nc.sync.dma_start(ident[:], ident_d[:, :])
            iota = cpool.tile([P, P], F32); nc.sync.dma_start(iota[:], iota_d[:, :])
            ones1 = cpool.tile([1, P], F32); nc.sync.dma_start(ones1[:], ones1_d[:, :])
            mlp1 = cpool.tile([F_IN + 1, HID], F32); nc.sync.dma_start(mlp1[:], mlp1_d[:, :])
            mlp2 = cpool.tile([HID + 1, HID], F32); nc.sync.dma_start(mlp2[:], mlp2_d[:, :])
            rhs1c = cpool.tile([73, HC + H], F32); nc.sync.dma_start(rhs1c[:], rhs1c_d[:, :])
            wr1 = cpool.tile([HID, HC + H], F32); nc.sync.dma_start(wr1[:], wr1_d[:, :])
            rhsxl1 = cpool.tile([P, HC], F32); nc.sync.dma_start(rhsxl1[:], rhsxl1_d[:, :])
            rhs2c = cpool.tile([9, HC + 1], F32); nc.sync.dma_start(rhs2c[:], rhs2c_d[:, :])
            wr2a = cpool.tile([P, HC + 1], F32); nc.sync.dma_start(wr2a[:], wr2a_d[:, :])
            wr2b = cpool.tile([P, HC + 1], F32); nc.sync.dma_start(wr2b[:], wr2b_d[:, :])
            wl2a = cpool.tile([P, HC + 1], F32); nc.sync.dma_start(wl2a[:], wl2a_d[:, :])
            wl2b = cpool.tile([P, HC + 1], F32); nc.sync.dma_start(wl2b[:], wl2b_d[:, :])
            xl2bias = cpool.tile([1, HC + 1], F32); nc.sync.dma_start(xl2bias[:], xl2bias_d[:, :])
            att2b = cpool.tile([P, HC], F32); nc.sync.dma_start(att2b[:], att2b_d[:, :])
            zeros = cpool.tile([P, HC + H], F32)
            nc.vector.memset(zeros[:], 0.0)

            # ---------------- phase 0: MLP encoder -> h_full [npad, 64] -----
            for i in range(nch0):
                sl = slice(i * 512, (i + 1) * 512)
                rx = kpool.tile([F_IN + 1, 512], F32, tag="mlp_rx")
                nc.sync.dma_start(rx[:], xt_d[:, sl])
                p1 = ps1.tile([HID, 512], F32, tag="mlp_ps")
                nc.tensor.matmul(p1[:], lhsT=mlp1[:], rhs=rx[:], start=True, stop=True)
                ht = kpool.tile([HID + 1, 512], F32, tag="mlp_ht")
                nc.scalar.activation(ht[0:HID, :], p1[:], AF.Relu)
                nc.vector.memset(ht[HID:HID + 1, :], 1.0)
                p2 = ps1.tile([HID, 512], F32, tag="mlp_ps")
                nc.tensor.matmul(p2[:], lhsT=mlp2[:], rhs=ht[:], start=True, stop=True)
                h2 = kpool.tile([HID, 512], F32, tag="mlp_h2")
                nc.scalar.activation(h2[:], p2[:], AF.Relu)
                hrow = kpool.tile([P, 4, HID], F32, tag="mlp_hrow")
                for j in range(4):
                    pt = ps1.tile([P, HID], F32, tag="pst")
                    nc.tensor.transpose(pt[:], h2[:, j * P:(j + 1) * P], ident[0:HID, 0:HID])
                    nc.scalar.activation(hrow[:, j, :], pt[:], AF.Copy)
                nc.sync.dma_start(
                    h_full[sl, :].rearrange("(j p) d -> p j d", p=P), hrow[:])

            # ---------------- phase 1: GAT layer 1 --------------------------
            ab_sl1 = [(h * HID, (h + 1) * HID) for h in range(H)]
            for w in range(nw1):
                span = min(w1, npc - w * w1)
                nb = w * w1
                # window rhs stack: [Wl1;We1;bias] scaled+aug (const) + xr rows
                hw = wpool.tile([w1, HID], F32, tag="hw")
                nc.sync.dma_start(hw[0:span, :], h_full[nb:nb + span, :])
                pt = ps1.tile([HID, w1], F32, tag="pst")
                nc.tensor.transpose(pt[:, 0:span], hw[0:span, :], ident[0:span, 0:span])
                hwT = wpool.tile([HID, w1], F32, tag="hwT")
                nc.scalar.activation(hwT[:, 0:span], pt[:, 0:span], AF.Copy)
                pxr = ps1.tile([w1, HC + H], F32, tag="ps_prep")
                nc.tensor.matmul(pxr[0:span, :], lhsT=hwT[:, 0:span], rhs=wr1[:],
                                 start=True, stop=True)
                rstk = wpool.tile([P, HC + H], F32, tag="rstk")
                nc.scalar.activation(rstk[0:73, :], rhs1c[:], AF.Copy)
                xrw = wpool.tile([w1, HC + H], F32, tag="xrw")
                nc.scalar.activation(xrw[0:span, :], pxr[0:span, :], AF.Copy)
                nc.sync.dma_start(rstk[73:73 + span, :], xrw[0:span, :])
                if span < w1:
                    nc.sync.dma_start(rstk[73 + span:P, :],
                                      zeros[0:w1 - span, 0:HC + H])

                srcw = wpool.tile([P, k1], I32, tag="srcw")
                nc.sync.dma_start(srcw[:], src1_d[:, w * k1:(w + 1) * k1])
                dstw = wpool.tile([P, k1], F32, tag="dstw")
                nc.sync.dma_start(dstw[:], dstl1_d[:, w * k1:(w + 1) * k1])
                pre = wpool.tile([P, k1, P], F32, tag="pre1")
                nc.sync.dma_start(
                    pre[:, :, HID:HID + ED],
                    ea1_d[:, w * k1 * ED:(w + 1) * k1 * ED].rearrange(
                        "p (k d) -> p k d", d=ED))
                nc.vector.memset(pre[:, :, 72:73], 1.0)
                for c in range(k1):
                    nc.gpsimd.indirect_dma_start(
                        out=pre[:, c, 0:HID], out_offset=None,
                        in_=h_full[:, :],
                        in_offset=IndirectOffsetOnAxis(ap=srcw[:, c:c + 1], axis=0))

                pout = ps2.tile([P, HC + H], F32, tag="ps_out")
                for c in range(k1):
                    nc.vector.tensor_scalar(
                        out=pre[:, c, 73:P], in0=iota[:, 0:55],
                        scalar1=dstw[:, c:c + 1], scalar2=None, op0=ALU.is_equal)
                    pt2 = ps1.tile([P, P], F32, tag="pst")
                    nc.tensor.transpose(pt2[:], pre[:, c, :], ident[:])
                    stk = kpool.tile([P, P], F32, tag="stk")
                    nc.scalar.activation(stk[:], pt2[:], AF.Copy)
                    pss = ps2.tile([P, HC + H], F32, tag="ps_s")
                    nc.tensor.matmul(pss[:], lhsT=stk[:], rhs=rstk[:], start=True, stop=True)
                    psx = ps1.tile([P, HC], F32, tag="ps_xl")
                    nc.tensor.matmul(psx[:], lhsT=stk[:], rhs=rhsxl1[:], start=True, stop=True)
                    scr = kpool.tile([P, HC], F32, tag="scr")
                    aabs = kpool.tile([P, 2 * H], F32, tag="aabs")
                    for h in range(H):
                        a, b = h * HID, (h + 1) * HID
                        mid = a + npos1[h]
                        if mid > a:
                            nc.scalar.activation(scr[:, a:mid], pss[:, a:mid], AF.Abs,
                                                 accum_out=aabs[:, h:h + 1])
                        else:
                            nc.vector.memset(aabs[:, h:h + 1], 0.0)
                        if b > mid:
                            nc.scalar.activation(scr[:, mid:b], pss[:, mid:b], AF.Abs,
                                                 accum_out=aabs[:, H + h:H + h + 1])
                        else:
                            nc.vector.memset(aabs[:, H + h:H + h + 1], 0.0)
                    t1 = kpool.tile([P, H], F32, tag="t1")
                    nc.vector.scalar_tensor_tensor(
                        out=t1[:], in0=aabs[:, 0:H], scalar=0.4,
                        in1=pss[:, HC:HC + H], op0=ALU.mult, op1=ALU.add)
                    alpha = kpool.tile([P, H], F32, tag="alpha")
                    nc.vector.scalar_tensor_tensor(
                        out=alpha[:], in0=aabs[:, H:2 * H], scalar=-0.4,
                        in1=t1[:], op0=ALU.mult, op1=ALU.add)
                    ex = kpool.tile([P, H], F32, tag="ex")
                    nc.scalar.activation(ex[:], alpha[:], AF.Exp)
                    pay = kpool.tile([P, HC + H], F32, tag="pay")
                    for h, (a, b) in enumerate(ab_sl1):
                        nc.vector.tensor_scalar(
                            out=pay[:, a:b], in0=psx[:, a:b],
                            scalar1=ex[:, h:h + 1], scalar2=None, op0=ALU.mult)
                    nc.vector.tensor_copy(pay[:, HC:HC + H], ex[:])
                    nc.tensor.matmul(pout[0:span, :], lhsT=pre[:, c, 73:73 + span],
                                     rhs=pay[:], start=(c == 0), stop=(c == k1 - 1))

                # normalize + relu -> h1 rows; also xl2 rows
                deng = wpool.tile([w1, H], F32, tag="deng")
                nc.vector.tensor_scalar(out=deng[0:span, :], in0=pout[0:span, HC:HC + H],
                                        scalar1=1e-30, scalar2=None, op0=ALU.max)
                rden = wpool.tile([w1, H], F32, tag="rden")
                nc.vector.reciprocal(rden[0:span, :], deng[0:span, :])
                h1w = wpool.tile([w1, HC], F32, tag="h1w")
                for h, (a, b) in enumerate(ab_sl1):
                    nc.vector.tensor_scalar(
                        out=h1w[0:span, a:b], in0=pout[0:span, a:b],
                        scalar1=rden[0:span, h:h + 1], scalar2=0.0,
                        op0=ALU.mult, op1=ALU.max)
                nc.sync.dma_start(h1loc[nb:nb + span, :], h1w[0:span, :])

                pxt = ps1.tile([P, w1], F32, tag="pst")
                h1T0 = wpool.tile([P, w1], F32, tag="h1T0")
                nc.tensor.transpose(pxt[:, 0:span], h1w[0:span, 0:P], ident[0:span, 0:span])
                nc.scalar.activation(h1T0[:, 0:span], pxt[:, 0:span], AF.Copy)
                pxt2 = ps1.tile([P, w1], F32, tag="pst")
                h1T1 = wpool.tile([P, w1], F32, tag="h1T1")
                nc.tensor.transpose(pxt2[:, 0:span], h1w[0:span, P:HC], ident[0:span, 0:span])
                nc.scalar.activation(h1T1[:, 0:span], pxt2[:, 0:span], AF.Copy)
                pxl2 = ps1.tile([w1, HC + 1], F32, tag="ps_prep")
                nc.tensor.matmul(pxl2[0:span, :], lhsT=h1T0[:, 0:span], rhs=wl2a[:],
                                 start=True, stop=False)
                nc.tensor.matmul(pxl2[0:span, :], lhsT=h1T1[:, 0:span], rhs=wl2b[:],
                                 start=False, stop=False)
                nc.tensor.matmul(pxl2[0:span, :], lhsT=ones1[:, 0:span], rhs=xl2bias[:],
                                 start=False, stop=True)
                xl2w = wpool.tile([w1, HC + 1], F32, tag="xl2w")
                nc.scalar.activation(xl2w[0:span, :], pxl2[0:span, :], AF.Copy)
                nc.sync.dma_start(xl2loc[nb:nb + span, :], xl2w[0:span, :])

            # ---------------- phase 2: allgather xl2 table -------------------
            nc.gpsimd.collective_compute(
                "AllGather", ALU.bypass,
                replica_groups=[list(range(NCORES))],
                ins=[xl2loc[:]], outs=[xl2full[:]])

            # ---------------- phase 3: GAT layer 2 --------------------------
            for w in range(nw2):
                span = min(w2, npc - w * w2)
                nb = w * w2
                h1r = wpool.tile([w2, HC], F32, tag="h1r")
                nc.sync.dma_start(h1r[0:span, :], h1loc[nb:nb + span, :])
                pt0 = ps1.tile([P, w2], F32, tag="pst")
                hrT0 = wpool.tile([P, w2], F32, tag="hrT0")
                nc.tensor.transpose(pt0[:, 0:span], h1r[0:span, 0:P], ident[0:span, 0:span])
                nc.scalar.activation(hrT0[:, 0:span], pt0[:, 0:span], AF.Copy)
                pt1 = ps1.tile([P, w2], F32, tag="pst")
                hrT1 = wpool.tile([P, w2], F32, tag="hrT1")
                nc.tensor.transpose(pt1[:, 0:span], h1r[0:span, P:HC], ident[0:span, 0:span])
                nc.scalar.activation(hrT1[:, 0:span], pt1[:, 0:span], AF.Copy)
                pxr2 = ps1.tile([w2, HC + 1], F32, tag="ps_prep")
                nc.tensor.matmul(pxr2[0:span, :], lhsT=hrT0[:, 0:span], rhs=wr2a[:],
                                 start=True, stop=False)
                nc.tensor.matmul(pxr2[0:span, :], lhsT=hrT1[:, 0:span], rhs=wr2b[:],
                                 start=False, stop=True)
                rstk2 = wpool.tile([P, HC + 1], F32, tag="rstk2")
                nc.scalar.activation(rstk2[0:9, :], rhs2c[:], AF.Copy)
                xrw2 = wpool.tile([w2, HC + 1], F32, tag="xrw2")
                nc.scalar.activation(xrw2[0:span, :], pxr2[0:span, :], AF.Copy)
                nc.sync.dma_start(rstk2[9:9 + span, :], xrw2[0:span, :])
                if span < w2:
                    nc.sync.dma_start(rstk2[9 + span:P, :],
                                      zeros[0:w2 - span, 0:HC + 1])

                srcw2 = wpool.tile([P, k2], I32, tag="srcw2")
                nc.sync.dma_start(srcw2[:], src2_d[:, w * k2:(w + 1) * k2])
                dstw2 = wpool.tile([P, k2], F32, tag="dstw2")
                nc.sync.dma_start(dstw2[:], dstl2_d[:, w * k2:(w + 1) * k2])
                pre2 = wpool.tile([P, k2, P], F32, tag="pre2")
                nc.sync.dma_start(
                    pre2[:, :, 0:ED],
                    ea2_d[:, w * k2 * ED:(w + 1) * k2 * ED].rearrange(
                        "p (k d) -> p k d", d=ED))
                nc.vector.memset(pre2[:, :, ED:ED + 1], 1.0)
                xg = wpool.tile([P, k2, HC + 1], F32, tag="xg")
                for c in range(k2):
                    nc.gpsimd.indirect_dma_start(
                        out=xg[:, c, :], out_offset=None,
                        in_=xl2full[:, :],
                        in_offset=IndirectOffsetOnAxis(ap=srcw2[:, c:c + 1], axis=0))

                pout2 = ps2.tile([P, HC + 1], F32, tag="ps_out")
                for c in range(k2):
                    nc.vector.tensor_scalar(
                        out=pre2[:, c, 9:P], in0=iota[:, 0:119],
                        scalar1=dstw2[:, c:c + 1], scalar2=None, op0=ALU.is_equal)
                    pt2 = ps1.tile([P, P], F32, tag="pst")
                    nc.tensor.transpose(pt2[:], pre2[:, c, :], ident[:])
                    stk2 = kpool.tile([P, P], F32, tag="stk")
                    nc.scalar.activation(stk2[:], pt2[:], AF.Copy)
                    pss2 = ps2.tile([P, HC + 1], F32, tag="ps_s")
                    nc.tensor.matmul(pss2[:], lhsT=stk2[:], rhs=rstk2[:],
                                     start=True, stop=False)
                    nc.tensor.matmul(pss2[:, 0:P], lhsT=ident[:], rhs=xg[:, c, 0:P],
                                     start=False, stop=False)
                    nc.tensor.matmul(pss2[:, P:HC], lhsT=ident[:], rhs=xg[:, c, P:HC],
                                     start=False, stop=True)
                    scr2 = kpool.tile([P, HC], F32, tag="scr")
                    nc.scalar.activation(scr2[:], pss2[:, 0:HC], AF.Abs)
                    wabs2 = kpool.tile([P, HC], F32, tag="wabs2")
                    aabs2 = kpool.tile([P, 1], F32, tag="aabs")
                    nc.vector.scalar_tensor_tensor(
                        out=wabs2[:], in0=scr2[:], scalar=1.0,
                        in1=att2b[:], op0=ALU.mult, op1=ALU.mult,
                        accum_out=aabs2[:])
                    alpha2 = kpool.tile([P, 1], F32, tag="alpha")
                    nc.vector.tensor_tensor(
                        out=alpha2[:], in0=aabs2[:], in1=pss2[:, HC:HC + 1],
                        op=ALU.add)
                    ex2 = kpool.tile([P, 1], F32, tag="ex")
                    nc.scalar.activation(ex2[:], alpha2[:], AF.Exp,
                                         bias=xg[:, c, HC:HC + 1], scale=1.0)
                    pay2 = kpool.tile([P, HC + 1], F32, tag="pay")
                    nc.vector.tensor_scalar(
                        out=pay2[:, 0:HC], in0=xg[:, c, 0:HC],
                        scalar1=ex2[:], scalar2=None, op0=ALU.mult)
                    nc.vector.tensor_copy(pay2[:, HC:HC + 1], ex2[:])
                    nc.tensor.matmul(pout2[0:span, :], lhsT=pre2[:, c, 9:9 + span],
                                     rhs=pay2[:], start=(c == 0), stop=(c == k2 - 1))

                deng2 = wpool.tile([w2, 1], F32, tag="deng")
                nc.vector.tensor_scalar(out=deng2[0:span, :], in0=pout2[0:span, HC:HC + 1],
                                        scalar1=1e-30, scalar2=None, op0=ALU.max)
                rden2 = wpool.tile([w2, 1], F32, tag="rden")
                nc.vector.reciprocal(rden2[0:span, :], deng2[0:span, :])
                outw = wpool.tile([w2, HC], F32, tag="outw")
                nc.vector.tensor_scalar(
                    out=outw[0:span, :], in0=pout2[0:span, 0:HC],
                    scalar1=rden2[0:span, :], scalar2=0.0, op0=ALU.mult, op1=ALU.max)
                nc.sync.dma_start(out_d[nb:nb + span, :], outw[0:span, :])

            if debug:
                nc.sync.dma_start(dbg_h[:, :], h_full[:, :])
                nc.sync.dma_start(dbg_h1[:, :], h1loc[:, :])
                nc.sync.dma_start(dbg_xf[:, :], xl2full[:, :])

    nc.finalize()
    return nc


# ----------------------------------------------------------------------------
# entry point
# ----------------------------------------------------------------------------

def _install_ntff_hook():
    """Shim antenv.axon_hooks so trace=True can collect NTFF profiles."""
    import types
    try:
        from antenv.axon_hooks import get_axon_ntff_profile_hook  # noqa: F401
        return
    except ImportError:
        pass
    try:
        import antenv
        boot_dir = "/root/.axon_site/trn_agent_boot"
        so_path = "/opt/axon/libaxon_pjrt.so"
        if boot_dir not in sys.path:
            sys.path.insert(0, boot_dir)
        import trn_boot
        mod = types.ModuleType("antenv.axon_hooks")
        _state = {"hook": None}
        mod.set_axon_ntff_profile_hook = lambda h: _state.__setitem__("hook", h)
        mod.get_axon_ntff_profile_hook = lambda: _state["hook"]
        sys.modules["antenv.axon_hooks"] = mod
        antenv.axon_hooks = mod
        if os.path.exists(so_path):
            mod.set_axon_ntff_profile_hook(
                trn_boot._ntff_profile_via_ctypes(so_path))
    except Exception as exc:  # profiling is best-effort
        print("ntff hook install failed:", exc)


def run(inputs, trace=False):
    if trace:
        _install_ntff_hook()
    n = int(inputs["x"].shape[0])
    e = int(inputs["edge_index"].shape[1])
    assert n % NCORES == 0
    npc = n // NCORES
    meta, in_maps = _prep_host(inputs, n, e, npc, w1=55, w2=119)
    nc = _build_nc(meta)
    res = run_bass_kernel_spmd(nc, in_maps, list(range(NCORES)), trace=trace)
    outs = [res.results[c]["out"] for c in range(NCORES)]
    full = np.concatenate(outs, 0).astype(np.float32)
    return full, res


def kernel(**inputs):
    full, _ = run(inputs, trace=False)
    return full


# revision 26
# speedup vs baseline: 1.4872x; 1.4872x over previous
"""GATv2FeatureExtractor Trainium2 kernel (8 NeuronCores, edge-parallel by dst).

Strategy
--------
Edges are sorted by destination and sharded into 8 contiguous dst ranges (6250
nodes per core).  Within each core the local node order is a degree-balanced
permutation so every fixed node-window (55 nodes for GAT layer 1, 119 for
layer 2) carries a near-equal edge count; the weighted message scatter is a
one-hot matmul accumulating into a PSUM window, so no cross-core reduction is
needed.  One collective runs: an AllGather of the layer-2 source table
(h1 @ Wl2) between the layers.  Gather indices are remapped on the host so the
SPMD program is identical on all cores.

GATv2 math is restructured so no per-edge softmax max pass is needed
(alpha is O(1) for this model; exp() cannot overflow), and
att . LReLU(s) = 0.6*att.s + 0.4*(sum_pos |att*s| - sum_neg |att*s|)
with the linear term riding as extra matmul columns and the |.| sums taken
per sign-group (host permutes columns per head by sign(att)).  All biases are
folded into matmul ones-rows; the attention scale factors into the payload.

Compute runs in bf16 (fp32 PSUM accumulation); the layer-2 exp bias column is
carried as a bf16 hi/lo pair to preserve precision.
"""

import os
import sys

import numpy as np

if os.path.isdir("/opt/trn_rl_repo") and "/opt/trn_rl_repo" not in sys.path:
    sys.path.insert(0, "/opt/trn_rl_repo")

import concourse.bacc as bacc
import concourse.bass as bass
import concourse.mybir as mybir
import concourse.tile as tile
from concourse.bass import IndirectOffsetOnAxis
from concourse.bass_utils import run_bass_kernel_spmd

F32 = mybir.dt.float32
BF16 = mybir.dt.float16  # 16-bit compute dtype (fp16: better mantissa, same speed)
I32 = mybir.dt.int32
AF = mybir.ActivationFunctionType
ALU = mybir.AluOpType

NCORES = 8
P = 128

F_IN, ED, HID, H, OUT = 32, 8, 64, 4, 256
HC = H * HID  # 256
XW = HC + 2   # xl2 table width: 256 payload + t_lin hi/lo

NPF = np.float32
NPB = "bfloat16"  # via ml_dtypes below

def _bf(a):
    """Cast numpy f32 array to the 16-bit compute dtype."""
    return np.asarray(a, np.float32).astype(np.float16)


def _ceil_div(a, b):
    return -(-a // b)


# ----------------------------------------------------------------------------
# host-side preprocessing
# ----------------------------------------------------------------------------

def _balanced_perms(deg, n, npc, span):
    """Per-core degree-balanced local permutation for `span`-node windows."""
    perms, invs = [], []
    for c in range(NCORES):
        d = deg[c * npc:(c + 1) * npc]
        order = np.argsort(-d, kind="stable")
        rows = np.arange(npc)
        seq = np.lexsort((rows // span, rows % span))
        perm = np.empty(npc, np.int64)
        perm[seq] = order
        inv = np.empty(npc, np.int64)
        inv[perm] = np.arange(npc)
        perms.append(perm)
        invs.append(inv)
    return perms, invs


def _pack_layer(src_row, dstl, winid, ea_e, nw, k):
    """Pack one core's edges (already remapped) into chunk-major arrays."""
    nchunk = nw * k
    src_f = np.zeros(nchunk * P, np.int32)
    dst_f = np.full(nchunk * P, -1.0, np.float32)
    ea_f = np.zeros((nchunk * P, ED), np.float32)
    order = np.argsort(winid, kind="stable")
    bounds = np.searchsorted(winid[order], np.arange(nw + 1))
    for w in range(nw):
        a, b = bounds[w], bounds[w + 1]
        m = b - a
        if m == 0:
            continue
        sel = order[a:b]
        pos = w * k * P + np.arange(m)
        src_f[pos] = src_row[sel]
        dst_f[pos] = dstl[sel]
        ea_f[pos] = ea_e[sel]
    src_a = src_f.reshape(nchunk, P).T.copy()
    dst_a = dst_f.reshape(nchunk, P).T.copy()
    ea_a = _bf(ea_f.reshape(nchunk, P, ED).transpose(1, 0, 2).reshape(P, nchunk * ED))
    return src_a, dst_a, ea_a


def _prep_host(inputs, n, e, npc, w1, w2):
    x = np.asarray(inputs["x"], np.float32)
    ei = np.asarray(inputs["edge_index"])
    ea = np.asarray(inputs["edge_attr"], np.float32)
    src = ei[0].astype(np.int64)
    dst = ei[1].astype(np.int64)

    deg = np.bincount(dst, minlength=n)
    perms, invs = _balanced_perms(deg, n, npc, w1)
    nw1 = _ceil_div(npc, w1)
    nw2 = _ceil_div(npc, w2)

    owner = dst // npc
    # per-core edge sets & window stats
    core_edges = [np.where(owner == c)[0] for c in range(NCORES)]
    k1 = k2 = 1
    core_pack = []
    for c in range(NCORES):
        es = core_edges[c]
        r = invs[c][dst[es] - c * npc]      # balanced local row of dst
        w1id = r // w1
        w2id = r // w2
        k1 = max(k1, int(np.bincount(w1id, minlength=nw1).max() or 1))
        k2 = max(k2, int(np.bincount(w2id, minlength=nw2).max() or 1))
        core_pack.append((es, r, w1id, w2id))
    k1 = _ceil_div(k1, P)
    k2 = _ceil_div(k2, P)

    inv_all = np.concatenate(invs)  # inv_all[o*npc + local_id] = local row in core o
    packed1, packed2 = [], []
    for c in range(NCORES):
        es, r, w1id, w2id = core_pack[c]
        lo = c * npc
        g = src[es]
        v = (g - lo) % n
        row1 = np.where(v < npc, invs[c][np.minimum(v, npc - 1)], v)  # L1 table row
        row2 = (g // npc) * npc + inv_all[g]                          # xl2full row
        packed1.append(_pack_layer(row1.astype(np.int64), (r - w1id * w1), w1id, ea[es], nw1, k1))
        packed2.append(_pack_layer(row2.astype(np.int64), (r - w2id * w2), w2id, ea[es], nw2, k2))

    # --- weights ---
    W1 = np.asarray(inputs["W1"], np.float32); b1 = np.asarray(inputs["b1"], np.float32)
    W2 = np.asarray(inputs["W2"], np.float32); b2 = np.asarray(inputs["b2"], np.float32)
    Wl1 = np.asarray(inputs["Wl1"], np.float32); bl1 = np.asarray(inputs["bl1"], np.float32)
    Wr1 = np.asarray(inputs["Wr1"], np.float32); br1 = np.asarray(inputs["br1"], np.float32)
    We1 = np.asarray(inputs["We1"], np.float32)
    att1 = np.asarray(inputs["att1"], np.float32)
    bias1 = np.asarray(inputs["bias1"], np.float32)
    Wl2 = np.asarray(inputs["Wl2"], np.float32); bl2 = np.asarray(inputs["bl2"], np.float32)
    Wr2 = np.asarray(inputs["Wr2"], np.float32); br2 = np.asarray(inputs["br2"], np.float32)
    We2 = np.asarray(inputs["We2"], np.float32)
    att2 = np.asarray(inputs["att2"], np.float32)
    bias2 = np.asarray(inputs["bias2"], np.float32)

    consts = {}
    consts["ident"] = _bf(np.eye(P, dtype=np.float32))
    consts["iota"] = _bf(np.tile(np.arange(P, dtype=np.float32), (P, 1)))
    consts["ones1"] = _bf(np.ones((1, P), np.float32))
    consts["mlp1"] = np.concatenate([W1, b1[None, :]], 0)
    consts["mlp2"] = np.concatenate([W2, b2[None, :]], 0)

    arow1 = att1.reshape(HC)
    abs1 = np.abs(arow1)
    amat1 = np.zeros((HC, H), np.float32)
    for h in range(H):
        amat1[h * HID:(h + 1) * HID, h] = att1[h]
    perm1 = np.concatenate([
        h * HID + np.concatenate([np.where(att1[h] > 0)[0], np.where(att1[h] <= 0)[0]])
        for h in range(H)]).astype(np.int64)
    npos1 = [int((att1[h] > 0).sum()) for h in range(H)]
    brow1 = (bl1 + br1)[None, :]

    def _aug1(m):
        return np.concatenate([(m * abs1[None, :])[:, perm1], 0.6 * (m @ amat1)], 1)

    consts["rhs1c"] = _bf(np.concatenate([_aug1(Wl1), _aug1(We1), _aug1(brow1)], 0))
    consts["wr1"] = _bf(_aug1(Wr1))
    rx = np.zeros((P, HC), np.float32)
    rx[0:HID] = Wl1
    rx[72] = bl1 + bias1
    consts["rhsxl1"] = _bf(rx)

    arow2 = att2.reshape(HC)

    def _aug2(m):
        return np.concatenate([m, 0.6 * (m @ arow2[:, None])], 1)

    brow2 = (br2 - bias2)[None, :]
    consts["rhs2c"] = _bf(np.concatenate([_aug2(We2), _aug2(brow2)], 0))
    wr2 = _aug2(Wr2)
    consts["wr2a"], consts["wr2b"] = _bf(wr2[0:P]), _bf(wr2[P:2 * P])
    wl2aug = np.concatenate([Wl2, 0.6 * (Wl2 @ arow2[:, None])], 1)
    consts["wl2a"], consts["wl2b"] = _bf(wl2aug[0:P]), _bf(wl2aug[P:2 * P])
    xb = (bl2 + bias2)[None, :]
    consts["xl2bias"] = _bf(np.concatenate([xb, 0.6 * (xb @ arow2[:, None]) - 4.0], 1))
    consts["att2b"] = _bf(np.tile(0.4 * arow2[None, :], (P, 1)))

    nch0 = _ceil_div(n, 512)
    npad = nch0 * 512
    xt_base = np.concatenate([x.T, np.ones((1, n), np.float32)], 0)

    in_maps = []
    for c in range(NCORES):
        lo = c * npc
        rot = np.concatenate([lo + perms[c], (lo + np.arange(npc, n)) % n])
        xt = np.zeros((F_IN + 1, npad), np.float32)
        xt[:, :n] = xt_base[:, rot]
        m = dict(consts)
        m["xt"] = xt
        m["src1"], m["dstl1"], m["ea1"] = packed1[c]
        m["src2"], m["dstl2"], m["ea2"] = packed2[c]
        in_maps.append(m)

    meta = dict(n=n, npc=npc, npad=npad, nch0=nch0,
                w1=w1, nw1=nw1, k1=int(k1), w2=w2, nw2=nw2, k2=int(k2),
                npos1=npos1)
    return meta, in_maps, perms


# ----------------------------------------------------------------------------
# device program
# ----------------------------------------------------------------------------

def _build_nc(meta, debug=False):
    n, npc, npad, nch0 = meta["n"], meta["npc"], meta["npad"], meta["nch0"]
    w1, nw1, k1 = meta["w1"], meta["nw1"], meta["k1"]
    w2, nw2, k2 = meta["w2"], meta["nw2"], meta["k2"]
    npos1 = meta["npos1"]

    nc = bacc.Bacc("TRN2", target_bir_lowering=False, num_devices=NCORES)

    def din(name, shape, dtype=BF16):
        return nc.dram_tensor(name, shape, dtype, kind="ExternalInput")

    ident_d = din("ident", [P, P])
    iota_d = din("iota", [P, P])
    ones1_d = din("ones1", [1, P])
    mlp1_d = din("mlp1", [F_IN + 1, HID], F32)
    mlp2_d = din("mlp2", [HID + 1, HID], F32)
    rhs1c_d = din("rhs1c", [73, HC + H])
    wr1_d = din("wr1", [HID, HC + H])
    rhsxl1_d = din("rhsxl1", [P, HC])
    rhs2c_d = din("rhs2c", [9, HC + 1])
    wr2a_d = din("wr2a", [P, HC + 1]); wr2b_d = din("wr2b", [P, HC + 1])
    wl2a_d = din("wl2a", [P, HC + 1]); wl2b_d = din("wl2b", [P, HC + 1])
    xl2bias_d = din("xl2bias", [1, HC + 1])
    att2b_d = din("att2b", [P, HC])
    xt_d = din("xt", [F_IN + 1, npad], F32)
    src1_d = din("src1", [P, nw1 * k1], I32)
    dstl1_d = din("dstl1", [P, nw1 * k1], F32)
    ea1_d = din("ea1", [P, nw1 * k1 * ED])
    src2_d = din("src2", [P, nw2 * k2], I32)
    dstl2_d = din("dstl2", [P, nw2 * k2], F32)
    ea2_d = din("ea2", [P, nw2 * k2 * ED])
    out_d = nc.dram_tensor("out", [npc, HC], F32, kind="ExternalOutput")
    if debug:
        dbg_h = nc.dram_tensor("dbg_h", [npad, HID], BF16, kind="ExternalOutput")
        dbg_h1 = nc.dram_tensor("dbg_h1", [npc, HC], BF16, kind="ExternalOutput")
        dbg_xf = nc.dram_tensor("dbg_xf", [NCORES * npc, XW], BF16, kind="ExternalOutput")

    with tile.TileContext(nc) as tc:
        with (
            tc.tile_pool(name="dram", bufs=1, space="DRAM") as dram,
            tc.tile_pool(name="const", bufs=1) as cpool,
            tc.tile_pool(name="win", bufs=2) as wpool,
            tc.tile_pool(name="chunk", bufs=3) as kpool,
            tc.tile_pool(name="ps2", bufs=2, space="PSUM") as ps2,
            tc.tile_pool(name="ps1", bufs=1, space="PSUM") as ps1,
        ):
            h_full = dram.tile([npad, HID], BF16)
            h1loc = dram.tile([npc, HC], BF16)
            xl2loc = dram.tile([npc, XW], BF16)
            xl2full = dram.tile([NCORES * npc, XW], BF16)

            def cload(name, shape, dt, src_d):
                t = cpool.tile(shape, dt, tag=name)
                nc.sync.dma_start(t[:], src_d[:, :])
                return t

            ident = cload("ident", [P, P], BF16, ident_d)
            iota = cload("iota", [P, P], BF16, iota_d)
            ones1 = cload("ones1", [1, P], BF16, ones1_d)
            mlp1 = cload("mlp1", [F_IN + 1, HID], F32, mlp1_d)
            mlp2 = cload("mlp2", [HID + 1, HID], F32, mlp2_d)
            rhs1c = cload("rhs1c", [73, HC + H], BF16, rhs1c_d)
            wr1 = cload("wr1", [HID, HC + H], BF16, wr1_d)
            rhsxl1 = cload("rhsxl1", [P, HC], BF16, rhsxl1_d)
            rhs2c = cload("rhs2c", [9, HC + 1], BF16, rhs2c_d)
            wr2a = cload("wr2a", [P, HC + 1], BF16, wr2a_d)
            wr2b = cload("wr2b", [P, HC + 1], BF16, wr2b_d)
            wl2a = cload("wl2a", [P, HC + 1], BF16, wl2a_d)
            wl2b = cload("wl2b", [P, HC + 1], BF16, wl2b_d)
            xl2bias = cload("xl2bias", [1, HC + 1], BF16, xl2bias_d)
            att2b = cload("att2b", [P, HC], BF16, att2b_d)
            zeros = cpool.tile([P, HC + H], BF16)
            nc.vector.memset(zeros[:], 0.0)
            neg4 = cpool.tile([P, 1], F32)
            nc.vector.memset(neg4[:], -4.0)

            # ---------------- phase 0: MLP encoder -> h_full ----------------
            for i in range(nch0):
                sl = slice(i * 512, (i + 1) * 512)
                rx = kpool.tile([F_IN + 1, 512], F32, tag="mlp_rx")
                nc.sync.dma_start(rx[:], xt_d[:, sl])
                p1 = ps1.tile([HID, 512], F32, tag="mlp_ps")
                nc.tensor.matmul(p1[:], lhsT=mlp1[:], rhs=rx[:], start=True, stop=True)
                ht = kpool.tile([HID + 1, 512], F32, tag="mlp_ht")
                nc.scalar.activation(ht[0:HID, :], p1[:], AF.Relu)
                nc.vector.memset(ht[HID:HID + 1, :], 1.0)
                p2 = ps1.tile([HID, 512], F32, tag="mlp_ps")
                nc.tensor.matmul(p2[:], lhsT=mlp2[:], rhs=ht[:], start=True, stop=True)
                h2 = kpool.tile([HID, 512], BF16, tag="mlp_h2")
                nc.scalar.activation(h2[:], p2[:], AF.Relu)
                hrow = kpool.tile([P, 4, HID], BF16, tag="mlp_hrow")
                for j in range(4):
                    pt = ps1.tile([P, HID], BF16, tag="pst")
                    nc.tensor.transpose(pt[:], h2[:, j * P:(j + 1) * P], ident[0:HID, 0:HID])
                    nc.scalar.activation(hrow[:, j, :], pt[:], AF.Copy)
                nc.sync.dma_start(
                    h_full[sl, :].rearrange("(j p) d -> p j d", p=P), hrow[:])

            # ---------------- phase 1: GAT layer 1 --------------------------
            ab_sl1 = [(h * HID, (h + 1) * HID) for h in range(H)]
            for w in range(nw1):
                span = min(w1, npc - w * w1)
                nb = w * w1
                hw = wpool.tile([w1, HID], BF16, tag="hw")
                nc.sync.dma_start(hw[0:span, :], h_full[nb:nb + span, :])
                pt = ps1.tile([HID, w1], BF16, tag="pst")
                nc.tensor.transpose(pt[:, 0:span], hw[0:span, :], ident[0:span, 0:span])
                hwT = wpool.tile([HID, w1], BF16, tag="hwT")
                nc.scalar.activation(hwT[:, 0:span], pt[:, 0:span], AF.Copy)
                pxr = ps1.tile([w1, HC + H], F32, tag="ps_prep")
                nc.tensor.matmul(pxr[0:span, :], lhsT=hwT[:, 0:span], rhs=wr1[:],
                                 start=True, stop=True)
                rstk = wpool.tile([P, HC + H], BF16, tag="rstk")
                nc.scalar.activation(rstk[0:73, :], rhs1c[:], AF.Copy)
                xrw = wpool.tile([w1, HC + H], BF16, tag="xrw")
                nc.scalar.activation(xrw[0:span, :], pxr[0:span, :], AF.Copy)
                nc.sync.dma_start(rstk[73:73 + span, :], xrw[0:span, :])
                if span < w1:
                    nc.sync.dma_start(rstk[73 + span:P, :],
                                      zeros[0:w1 - span, 0:HC + H])

                srcw = wpool.tile([P, k1], I32, tag="srcw")
                nc.sync.dma_start(srcw[:], src1_d[:, w * k1:(w + 1) * k1])
                dstw = wpool.tile([P, k1], F32, tag="dstw")
                nc.sync.dma_start(dstw[:], dstl1_d[:, w * k1:(w + 1) * k1])
                pre = wpool.tile([P, k1, P], BF16, tag="pre1")
                nc.sync.dma_start(
                    pre[:, :, HID:HID + ED],
                    ea1_d[:, w * k1 * ED:(w + 1) * k1 * ED].rearrange(
                        "p (k d) -> p k d", d=ED))
                nc.vector.memset(pre[:, :, 72:73], 1.0)
                for c in range(k1):
                    nc.vector.tensor_scalar(
                        out=pre[:, c, 73:P], in0=iota[:, 0:55],
                        scalar1=dstw[:, c:c + 1], scalar2=None, op0=ALU.is_equal)
                for c in range(k1):
                    nc.gpsimd.indirect_dma_start(
                        out=pre[:, c, 0:HID], out_offset=None,
                        in_=h_full[:, :],
                        in_offset=IndirectOffsetOnAxis(ap=srcw[:, c:c + 1], axis=0))

                pout = ps2.tile([P, HC + H], F32, tag="ps_out")
                for c in range(k1):
                    pt2 = ps1.tile([P, P], BF16, tag="pst")
                    nc.tensor.transpose(pt2[:], pre[:, c, :], ident[:])
                    stk = kpool.tile([P, P], BF16, tag="stk")
                    nc.scalar.activation(stk[:], pt2[:], AF.Copy)
                    pss = ps2.tile([P, HC + H], F32, tag="ps_s")
                    nc.tensor.matmul(pss[:], lhsT=stk[:], rhs=rstk[:], start=True, stop=True)
                    psx = ps1.tile([P, HC], F32, tag="ps_xl")
                    nc.tensor.matmul(psx[:], lhsT=stk[:], rhs=rhsxl1[:], start=True, stop=True)
                    abss = kpool.tile([P, HC], BF16, tag="abss")
                    nc.scalar.activation(abss[:], pss[:, 0:HC], AF.Abs)
                    scr = kpool.tile([P, HC], BF16, tag="scr")
                    aabs = kpool.tile([P, 2 * H], F32, tag="aabs")
                    for h in range(H):
                        a, b = h * HID, (h + 1) * HID
                        mid = a + npos1[h]
                        if mid > a:
                            nc.vector.tensor_scalar(
                                out=scr[:, a:mid], in0=abss[:, a:mid], scalar1=1.0,
                                scalar2=None, op0=ALU.mult, op1=ALU.add,
                                accum_out=aabs[:, h:h + 1])
                        else:
                            nc.vector.memset(aabs[:, h:h + 1], 0.0)
                        if b > mid:
                            nc.vector.tensor_scalar(
                                out=scr[:, mid:b], in0=abss[:, mid:b], scalar1=1.0,
                                scalar2=None, op0=ALU.mult, op1=ALU.add,
                                accum_out=aabs[:, H + h:H + h + 1])
                        else:
                            nc.vector.memset(aabs[:, H + h:H + h + 1], 0.0)
                    t1 = kpool.tile([P, H], F32, tag="t1")
                    nc.vector.scalar_tensor_tensor(
                        out=t1[:], in0=aabs[:, 0:H], scalar=0.4,
                        in1=pss[:, HC:HC + H], op0=ALU.mult, op1=ALU.add)
                    alpha = kpool.tile([P, H], F32, tag="alpha")
                    nc.vector.scalar_tensor_tensor(
                        out=alpha[:], in0=aabs[:, H:2 * H], scalar=-0.4,
                        in1=t1[:], op0=ALU.mult, op1=ALU.add)
                    ex = kpool.tile([P, H], F32, tag="ex")
                    nc.scalar.activation(ex[:], alpha[:], AF.Exp, bias=neg4[:])
                    pay = kpool.tile([P, HC + H], BF16, tag="pay")
                    for h, (a, b) in enumerate(ab_sl1):
                        nc.scalar.activation(pay[:, a:b], psx[:, a:b], AF.Copy,
                                             scale=ex[:, h:h + 1])
                    nc.scalar.activation(pay[:, HC:HC + H], ex[:], AF.Copy)
                    nc.tensor.matmul(pout[0:span, :], lhsT=pre[:, c, 73:73 + span],
                                     rhs=pay[:], start=(c == 0), stop=(c == k1 - 1))

                deng = wpool.tile([w1, H], F32, tag="deng")
                nc.vector.tensor_scalar(out=deng[0:span, :], in0=pout[0:span, HC:HC + H],
                                        scalar1=1e-30, scalar2=None, op0=ALU.max)
                rden = wpool.tile([w1, H], F32, tag="rden")
                nc.vector.reciprocal(rden[0:span, :], deng[0:span, :])
                h1w = wpool.tile([w1, HC], BF16, tag="h1w")
                for h, (a, b) in enumerate(ab_sl1):
                    nc.vector.tensor_scalar(
                        out=h1w[0:span, a:b], in0=pout[0:span, a:b],
                        scalar1=rden[0:span, h:h + 1], scalar2=0.0,
                        op0=ALU.mult, op1=ALU.max)
                nc.sync.dma_start(h1loc[nb:nb + span, :], h1w[0:span, :])

                pxt = ps1.tile([P, w1], BF16, tag="pst")
                h1T0 = wpool.tile([P, w1], BF16, tag="h1T0")
                nc.tensor.transpose(pxt[:, 0:span], h1w[0:span, 0:P], ident[0:span, 0:span])
                nc.scalar.activation(h1T0[:, 0:span], pxt[:, 0:span], AF.Copy)
                pxt2 = ps1.tile([P, w1], BF16, tag="pst")
                h1T1 = wpool.tile([P, w1], BF16, tag="h1T1")
                nc.tensor.transpose(pxt2[:, 0:span], h1w[0:span, P:HC], ident[0:span, 0:span])
                nc.scalar.activation(h1T1[:, 0:span], pxt2[:, 0:span], AF.Copy)
                pxl2 = ps1.tile([w1, HC + 1], F32, tag="ps_prep")
                nc.tensor.matmul(pxl2[0:span, :], lhsT=h1T0[:, 0:span], rhs=wl2a[:],
                                 start=True, stop=False)
                nc.tensor.matmul(pxl2[0:span, :], lhsT=h1T1[:, 0:span], rhs=wl2b[:],
                                 start=False, stop=False)
                nc.tensor.matmul(pxl2[0:span, :], lhsT=ones1[:, 0:span], rhs=xl2bias[:],
                                 start=False, stop=True)
                xl2w = wpool.tile([w1, XW], BF16, tag="xl2w")
                nc.scalar.activation(xl2w[0:span, 0:HC + 1], pxl2[0:span, :], AF.Copy)
                # lo residual of the t_lin column (bf16 hi/lo pair)
                tl_lo = wpool.tile([w1, 1], BF16, tag="tl_lo")
                nc.vector.tensor_tensor(
                    out=tl_lo[0:span, :], in0=pxl2[0:span, HC:HC + 1],
                    in1=xl2w[0:span, HC:HC + 1], op=ALU.subtract)
                nc.vector.tensor_copy(xl2w[0:span, HC + 1:XW], tl_lo[0:span, :])
                nc.sync.dma_start(xl2loc[nb:nb + span, :], xl2w[0:span, :])

            # ---------------- phase 2: allgather xl2 table -------------------
            nc.gpsimd.collective_compute(
                "AllGather", ALU.bypass,
                replica_groups=[list(range(NCORES))],
                ins=[xl2loc[:]], outs=[xl2full[:]])

            # ---------------- phase 3: GAT layer 2 --------------------------
            for w in range(nw2):
                span = min(w2, npc - w * w2)
                nb = w * w2
                h1r = wpool.tile([w2, HC], BF16, tag="h1r")
                nc.sync.dma_start(h1r[0:span, :], h1loc[nb:nb + span, :])
                pt0 = ps1.tile([P, w2], BF16, tag="pst")
                hrT0 = wpool.tile([P, w2], BF16, tag="hrT0")
                nc.tensor.transpose(pt0[:, 0:span], h1r[0:span, 0:P], ident[0:span, 0:span])
                nc.scalar.activation(hrT0[:, 0:span], pt0[:, 0:span], AF.Copy)
                pt1 = ps1.tile([P, w2], BF16, tag="pst")
                hrT1 = wpool.tile([P, w2], BF16, tag="hrT1")
                nc.tensor.transpose(pt1[:, 0:span], h1r[0:span, P:HC], ident[0:span, 0:span])
                nc.scalar.activation(hrT1[:, 0:span], pt1[:, 0:span], AF.Copy)
                pxr2 = ps1.tile([w2, HC + 1], F32, tag="ps_prep")
                nc.tensor.matmul(pxr2[0:span, :], lhsT=hrT0[:, 0:span], rhs=wr2a[:],
                                 start=True, stop=False)
                nc.tensor.matmul(pxr2[0:span, :], lhsT=hrT1[:, 0:span], rhs=wr2b[:],
                                 start=False, stop=True)
                rstk2 = wpool.tile([P, HC + 1], BF16, tag="rstk2")
                nc.scalar.activation(rstk2[0:9, :], rhs2c[:], AF.Copy)
                xrw2 = wpool.tile([w2, HC + 1], BF16, tag="xrw2")
                nc.scalar.activation(xrw2[0:span, :], pxr2[0:span, :], AF.Copy)
                nc.sync.dma_start(rstk2[9:9 + span, :], xrw2[0:span, :])
                if span < w2:
                    nc.sync.dma_start(rstk2[9 + span:P, :],
                                      zeros[0:w2 - span, 0:HC + 1])

                srcw2 = wpool.tile([P, k2], I32, tag="srcw2")
                nc.sync.dma_start(srcw2[:], src2_d[:, w * k2:(w + 1) * k2])
                dstw2 = wpool.tile([P, k2], F32, tag="dstw2")
                nc.sync.dma_start(dstw2[:], dstl2_d[:, w * k2:(w + 1) * k2])
                pre2 = wpool.tile([P, k2, P], BF16, tag="pre2")
                nc.sync.dma_start(
                    pre2[:, :, 0:ED],
                    ea2_d[:, w * k2 * ED:(w + 1) * k2 * ED].rearrange(
                        "p (k d) -> p k d", d=ED))
                nc.vector.memset(pre2[:, :, ED:ED + 1], 1.0)
                for c in range(k2):
                    nc.vector.tensor_scalar(
                        out=pre2[:, c, 9:P], in0=iota[:, 0:119],
                        scalar1=dstw2[:, c:c + 1], scalar2=None, op0=ALU.is_equal)
                xg = wpool.tile([P, k2, XW], BF16, tag="xg")
                for c in range(k2):
                    nc.gpsimd.indirect_dma_start(
                        out=xg[:, c, :], out_offset=None,
                        in_=xl2full[:, :],
                        in_offset=IndirectOffsetOnAxis(ap=srcw2[:, c:c + 1], axis=0))

                pout2 = ps2.tile([P, HC + 1], F32, tag="ps_out")
                for c in range(k2):
                    pt2 = ps1.tile([P, P], BF16, tag="pst")
                    nc.tensor.transpose(pt2[:], pre2[:, c, :], ident[:])
                    stk2 = kpool.tile([P, P], BF16, tag="stk")
                    nc.scalar.activation(stk2[:], pt2[:], AF.Copy)
                    pss2 = ps2.tile([P, HC + 1], F32, tag="ps_s")
                    nc.tensor.matmul(pss2[:], lhsT=stk2[:], rhs=rstk2[:],
                                     start=True, stop=False)
                    nc.tensor.matmul(pss2[:, 0:P], lhsT=ident[:], rhs=xg[:, c, 0:P],
                                     start=False, stop=False)
                    nc.tensor.matmul(pss2[:, P:HC], lhsT=ident[:], rhs=xg[:, c, P:HC],
                                     start=False, stop=True)
                    scr2 = kpool.tile([P, HC], BF16, tag="abss")
                    nc.scalar.activation(scr2[:], pss2[:, 0:HC], AF.Abs)
                    wabs2 = kpool.tile([P, HC], BF16, tag="scr")
                    aabs2 = kpool.tile([P, 1], F32, tag="aabs")
                    nc.vector.scalar_tensor_tensor(
                        out=wabs2[:], in0=scr2[:], scalar=1.0,
                        in1=att2b[:], op0=ALU.mult, op1=ALU.mult,
                        accum_out=aabs2[:])
                    alpha2 = kpool.tile([P, 1], F32, tag="alpha")
                    nc.vector.tensor_tensor(
                        out=alpha2[:], in0=aabs2[:], in1=pss2[:, HC:HC + 1],
                        op=ALU.add)
                    tlin = kpool.tile([P, 1], F32, tag="t1")
                    nc.vector.tensor_tensor(
                        out=tlin[:], in0=xg[:, c, HC:HC + 1],
                        in1=xg[:, c, HC + 1:XW], op=ALU.add)
                    ex2 = kpool.tile([P, 1], F32, tag="ex")
                    nc.scalar.activation(ex2[:], alpha2[:], AF.Exp,
                                         bias=tlin[:], scale=1.0)
                    pay2 = kpool.tile([P, HC + 1], BF16, tag="pay")
                    nc.scalar.activation(pay2[:, 0:HC], xg[:, c, 0:HC], AF.Copy,
                                         scale=ex2[:])
                    nc.scalar.activation(pay2[:, HC:HC + 1], ex2[:], AF.Copy)
                    nc.tensor.matmul(pout2[0:span, :], lhsT=pre2[:, c, 9:9 + span],
                                     rhs=pay2[:], start=(c == 0), stop=(c == k2 - 1))

                deng2 = wpool.tile([w2, 1], F32, tag="deng")
                nc.vector.tensor_scalar(out=deng2[0:span, :], in0=pout2[0:span, HC:HC + 1],
                                        scalar1=1e-30, scalar2=None, op0=ALU.max)
                rden2 = wpool.tile([w2, 1], F32, tag="rden")
                nc.vector.reciprocal(rden2[0:span, :], deng2[0:span, :])
                outw = wpool.tile([w2, HC], F32, tag="outw")
                nc.vector.tensor_scalar(
                    out=outw[0:span, :], in0=pout2[0:span, 0:HC],
                    scalar1=rden2[0:span, :], scalar2=0.0, op0=ALU.mult, op1=ALU.max)
                nc.sync.dma_start(out_d[nb:nb + span, :], outw[0:span, :])

            if debug:
                nc.sync.dma_start(dbg_h[:, :], h_full[:, :])
                nc.sync.dma_start(dbg_h1[:, :], h1loc[:, :])
                nc.sync.dma_start(dbg_xf[:, :], xl2full[:, :])

    nc.finalize()
    return nc


# ----------------------------------------------------------------------------
# entry point
# ----------------------------------------------------------------------------

def _install_ntff_hook():
    """Shim antenv.axon_hooks so trace=True can collect NTFF profiles."""
    import types
    try:
        from antenv.axon_hooks import get_axon_ntff_profile_hook  # noqa: F401
        return
    except ImportError:
        pass
    try:
        import antenv
        boot_dir = "/root/.axon_site/trn_agent_boot"
        so_path = "/opt/axon/libaxon_pjrt.so"
        if boot_dir not in sys.path:
            sys.path.insert(0, boot_dir)
        import trn_boot
        mod = types.ModuleType("antenv.axon_hooks")
        _state = {"hook": None}
        mod.set_axon_ntff_profile_hook = lambda h: _state.__setitem__("hook", h)
        mod.get_axon_ntff_profile_hook = lambda: _state["hook"]
        sys.modules["antenv.axon_hooks"] = mod
        antenv.axon_hooks = mod
        if os.path.exists(so_path):
            mod.set_axon_ntff_profile_hook(
                trn_boot._ntff_profile_via_ctypes(so_path))
    except Exception as exc:  # profiling is best-effort
        print("ntff hook install failed:", exc)


def run(inputs, trace=False):
    if trace:
        _install_ntff_hook()
    n = int(inputs["x"].shape[0])
    e = int(inputs["edge_index"].shape[1])
    assert n % NCORES == 0
    npc = n // NCORES
    meta, in_maps, perms = _prep_host(inputs, n, e, npc, w1=55, w2=119)
    nc = _build_nc(meta)
    res = run_bass_kernel_spmd(nc, in_maps, list(range(NCORES)), trace=trace)
    full = np.empty((n, HC), np.float32)
    for c in range(NCORES):
        full[c * npc + perms[c]] = res.results[c]["out"]
    return full, res


def kernel(**inputs):
    full, _ = run(inputs, trace=False)
    return full


# revision 28
# speedup vs baseline: 1.7207x; 1.1570x over previous
"""GATv2FeatureExtractor Trainium2 kernel (8 NeuronCores, edge-parallel by dst).

Strategy
--------
Edges are sorted by destination and sharded into 8 contiguous dst ranges (6250
nodes per core).  Within each core the local node order is a degree-balanced
permutation so every fixed node-window (55 nodes for GAT layer 1, 119 for
layer 2) carries a near-equal edge count; the weighted message scatter is a
one-hot matmul accumulating into a PSUM window, so no cross-core reduction is
needed.  One collective runs: an AllGather of the layer-2 source table
(h1 @ Wl2) between the layers.  Gather indices are remapped on the host so the
SPMD program is identical on all cores.

GATv2 math is restructured so no per-edge softmax max pass is needed
(alpha is O(1) for this model; exp() cannot overflow), and
att . LReLU(s) = 0.6*att.s + 0.4*(sum_pos |att*s| - sum_neg |att*s|)
with the linear term riding as extra matmul columns and the |.| sums taken
per sign-group (host permutes columns per head by sign(att)).  All biases are
folded into matmul ones-rows; the attention scale factors into the payload.

Compute runs in bf16 (fp32 PSUM accumulation); the layer-2 exp bias column is
carried as a bf16 hi/lo pair to preserve precision.
"""

import os
import sys

import numpy as np

if os.path.isdir("/opt/trn_rl_repo") and "/opt/trn_rl_repo" not in sys.path:
    sys.path.insert(0, "/opt/trn_rl_repo")

import concourse.bacc as bacc
import concourse.bass as bass
import concourse.mybir as mybir
import concourse.tile as tile
from concourse.bass import IndirectOffsetOnAxis
from concourse.bass_utils import run_bass_kernel_spmd

F32 = mybir.dt.float32
BF16 = mybir.dt.float16  # 16-bit compute dtype (fp16: better mantissa, same speed)
I32 = mybir.dt.int32
AF = mybir.ActivationFunctionType
ALU = mybir.AluOpType

NCORES = 8
P = 128

F_IN, ED, HID, H, OUT = 32, 8, 64, 4, 256
HC = H * HID  # 256
XW = HC + 2   # xl2 table width: 256 payload + t_lin hi/lo

NPF = np.float32
NPB = "bfloat16"  # via ml_dtypes below

def _bf(a):
    """Cast numpy f32 array to the 16-bit compute dtype."""
    return np.asarray(a, np.float32).astype(np.float16)


def _ceil_div(a, b):
    return -(-a // b)


# ----------------------------------------------------------------------------
# host-side preprocessing
# ----------------------------------------------------------------------------

def _balanced_perms(deg, n, npc, span):
    """Per-core degree-balanced local permutation for `span`-node windows."""
    perms, invs = [], []
    for c in range(NCORES):
        d = deg[c * npc:(c + 1) * npc]
        order = np.argsort(-d, kind="stable")
        rows = np.arange(npc)
        seq = np.lexsort((rows // span, rows % span))
        perm = np.empty(npc, np.int64)
        perm[seq] = order
        inv = np.empty(npc, np.int64)
        inv[perm] = np.arange(npc)
        perms.append(perm)
        invs.append(inv)
    return perms, invs


def _pack_layer(src_row, dstl, winid, ea_e, nw, k):
    """Pack one core's edges (already remapped) into chunk-major arrays."""
    nchunk = nw * k
    src_f = np.zeros(nchunk * P, np.int32)
    dst_f = np.full(nchunk * P, -1.0, np.float32)
    ea_f = np.zeros((nchunk * P, ED), np.float32)
    order = np.argsort(winid, kind="stable")
    bounds = np.searchsorted(winid[order], np.arange(nw + 1))
    for w in range(nw):
        a, b = bounds[w], bounds[w + 1]
        m = b - a
        if m == 0:
            continue
        sel = order[a:b]
        pos = w * k * P + np.arange(m)
        src_f[pos] = src_row[sel]
        dst_f[pos] = dstl[sel]
        ea_f[pos] = ea_e[sel]
    src_a = src_f.reshape(nchunk, P).T.copy()
    dst_a = dst_f.reshape(nchunk, P).T.copy()
    ea_a = _bf(ea_f.reshape(nchunk, P, ED).transpose(1, 0, 2).reshape(P, nchunk * ED))
    return src_a, dst_a, ea_a


def _prep_host(inputs, n, e, npc, w1, w2):
    x = np.asarray(inputs["x"], np.float32)
    ei = np.asarray(inputs["edge_index"])
    ea = np.asarray(inputs["edge_attr"], np.float32)
    src = ei[0].astype(np.int64)
    dst = ei[1].astype(np.int64)

    deg = np.bincount(dst, minlength=n)
    perms, invs = _balanced_perms(deg, n, npc, w1)
    nw1 = _ceil_div(npc, w1)
    nw2 = _ceil_div(npc, w2)

    owner = dst // npc
    # per-core edge sets & window stats
    core_edges = [np.where(owner == c)[0] for c in range(NCORES)]
    k1 = k2 = 1
    core_pack = []
    for c in range(NCORES):
        es = core_edges[c]
        r = invs[c][dst[es] - c * npc]      # balanced local row of dst
        w1id = r // w1
        w2id = r // w2
        k1 = max(k1, int(np.bincount(w1id, minlength=nw1).max() or 1))
        k2 = max(k2, int(np.bincount(w2id, minlength=nw2).max() or 1))
        core_pack.append((es, r, w1id, w2id))
    k1 = _ceil_div(k1, P)
    k2 = _ceil_div(k2, P)

    inv_all = np.concatenate(invs)  # inv_all[o*npc + local_id] = local row in core o
    packed1, packed2 = [], []
    for c in range(NCORES):
        es, r, w1id, w2id = core_pack[c]
        lo = c * npc
        g = src[es]
        v = (g - lo) % n
        row1 = np.where(v < npc, invs[c][np.minimum(v, npc - 1)], v)  # L1 table row
        row2 = (g // npc) * npc + inv_all[g]                          # xl2full row
        packed1.append(_pack_layer(row1.astype(np.int64), (r - w1id * w1), w1id, ea[es], nw1, k1))
        packed2.append(_pack_layer(row2.astype(np.int64), (r - w2id * w2), w2id, ea[es], nw2, k2))

    # --- weights ---
    W1 = np.asarray(inputs["W1"], np.float32); b1 = np.asarray(inputs["b1"], np.float32)
    W2 = np.asarray(inputs["W2"], np.float32); b2 = np.asarray(inputs["b2"], np.float32)
    Wl1 = np.asarray(inputs["Wl1"], np.float32); bl1 = np.asarray(inputs["bl1"], np.float32)
    Wr1 = np.asarray(inputs["Wr1"], np.float32); br1 = np.asarray(inputs["br1"], np.float32)
    We1 = np.asarray(inputs["We1"], np.float32)
    att1 = np.asarray(inputs["att1"], np.float32)
    bias1 = np.asarray(inputs["bias1"], np.float32)
    Wl2 = np.asarray(inputs["Wl2"], np.float32); bl2 = np.asarray(inputs["bl2"], np.float32)
    Wr2 = np.asarray(inputs["Wr2"], np.float32); br2 = np.asarray(inputs["br2"], np.float32)
    We2 = np.asarray(inputs["We2"], np.float32)
    att2 = np.asarray(inputs["att2"], np.float32)
    bias2 = np.asarray(inputs["bias2"], np.float32)

    consts = {}
    consts["ident"] = _bf(np.eye(P, dtype=np.float32))
    consts["iota"] = _bf(np.tile(np.arange(P, dtype=np.float32), (P, 1)))
    consts["ones1"] = _bf(np.ones((1, P), np.float32))
    consts["mlp1"] = _bf(np.concatenate([W1, b1[None, :]], 0))
    consts["mlp2"] = _bf(np.concatenate([W2, b2[None, :]], 0))

    arow1 = att1.reshape(HC)
    abs1 = np.abs(arow1)
    amat1 = np.zeros((HC, H), np.float32)
    for h in range(H):
        amat1[h * HID:(h + 1) * HID, h] = att1[h]
    perm1 = np.concatenate([
        h * HID + np.concatenate([np.where(att1[h] > 0)[0], np.where(att1[h] <= 0)[0]])
        for h in range(H)]).astype(np.int64)
    npos1 = [int((att1[h] > 0).sum()) for h in range(H)]
    brow1 = (bl1 + br1)[None, :]

    def _aug1(m):
        return np.concatenate([(m * abs1[None, :])[:, perm1], 0.6 * (m @ amat1)], 1)

    r1c = np.zeros((P, HC + H), np.float32)
    r1c[0:HID] = _aug1(Wl1)
    r1c[HID:HID + ED] = _aug1(We1)
    r1c[127] = _aug1(brow1)[0]
    consts["rhs1c"] = _bf(r1c)  # full 128 rows; Dsel rows 72:127 overwritten per window
    consts["wr1"] = _bf(_aug1(Wr1))
    rx = np.zeros((P, HC), np.float32)
    rx[0:HID] = Wl1
    rx[127] = bl1 + bias1
    consts["rhsxl1"] = _bf(rx)

    arow2 = att2.reshape(HC)

    def _aug2(m):
        return np.concatenate([m, 0.6 * (m @ arow2[:, None])], 1)

    brow2 = (br2 - bias2)[None, :]
    r2c = np.zeros((P, HC + 1), np.float32)
    r2c[0:ED] = _aug2(We2)
    r2c[127] = _aug2(brow2)[0]
    consts["rhs2c"] = _bf(r2c)
    wr2 = _aug2(Wr2)
    consts["wr2a"], consts["wr2b"] = _bf(wr2[0:P]), _bf(wr2[P:2 * P])
    wl2aug = np.concatenate([Wl2, 0.6 * (Wl2 @ arow2[:, None])], 1)
    consts["wl2a"], consts["wl2b"] = _bf(wl2aug[0:P]), _bf(wl2aug[P:2 * P])
    xb = (bl2 + bias2)[None, :]
    consts["xl2bias"] = _bf(np.concatenate([xb, 0.6 * (xb @ arow2[:, None]) - 4.0], 1))
    consts["att2b"] = _bf(np.tile(0.4 * arow2[None, :], (P, 1)))

    nch0 = _ceil_div(n, 512)
    npad = nch0 * 512
    xt_base = np.concatenate([x.T, np.ones((1, n), np.float32)], 0)

    in_maps = []
    for c in range(NCORES):
        lo = c * npc
        rot = np.concatenate([lo + perms[c], (lo + np.arange(npc, n)) % n])
        xt = np.zeros((F_IN + 1, npad), np.float16)
        xt[:, :n] = _bf(xt_base[:, rot])
        m = dict(consts)
        m["xt"] = xt
        m["src1"], m["dstl1"], m["ea1"] = packed1[c]
        m["src2"], m["dstl2"], m["ea2"] = packed2[c]
        in_maps.append(m)

    meta = dict(n=n, npc=npc, npad=npad, nch0=nch0,
                w1=w1, nw1=nw1, k1=int(k1), w2=w2, nw2=nw2, k2=int(k2),
                npos1=npos1)
    return meta, in_maps, perms


# ----------------------------------------------------------------------------
# device program
# ----------------------------------------------------------------------------

def _build_nc(meta, debug=False):
    n, npc, npad, nch0 = meta["n"], meta["npc"], meta["npad"], meta["nch0"]
    w1, nw1, k1 = meta["w1"], meta["nw1"], meta["k1"]
    w2, nw2, k2 = meta["w2"], meta["nw2"], meta["k2"]
    npos1 = meta["npos1"]

    nc = bacc.Bacc("TRN2", target_bir_lowering=False, num_devices=NCORES)

    def din(name, shape, dtype=BF16):
        return nc.dram_tensor(name, shape, dtype, kind="ExternalInput")

    ident_d = din("ident", [P, P])
    iota_d = din("iota", [P, P])
    ones1_d = din("ones1", [1, P])
    mlp1_d = din("mlp1", [F_IN + 1, HID])
    mlp2_d = din("mlp2", [HID + 1, HID])
    rhs1c_d = din("rhs1c", [P, HC + H])
    wr1_d = din("wr1", [HID, HC + H])
    rhsxl1_d = din("rhsxl1", [P, HC])
    rhs2c_d = din("rhs2c", [P, HC + 1])
    wr2a_d = din("wr2a", [P, HC + 1]); wr2b_d = din("wr2b", [P, HC + 1])
    wl2a_d = din("wl2a", [P, HC + 1]); wl2b_d = din("wl2b", [P, HC + 1])
    xl2bias_d = din("xl2bias", [1, HC + 1])
    att2b_d = din("att2b", [P, HC])
    xt_d = din("xt", [F_IN + 1, npad])
    src1_d = din("src1", [P, nw1 * k1], I32)
    dstl1_d = din("dstl1", [P, nw1 * k1], F32)
    ea1_d = din("ea1", [P, nw1 * k1 * ED])
    src2_d = din("src2", [P, nw2 * k2], I32)
    dstl2_d = din("dstl2", [P, nw2 * k2], F32)
    ea2_d = din("ea2", [P, nw2 * k2 * ED])
    out_d = nc.dram_tensor("out", [npc, HC], F32, kind="ExternalOutput")
    if debug:
        dbg_h = nc.dram_tensor("dbg_h", [npad, HID], BF16, kind="ExternalOutput")
        dbg_h1 = nc.dram_tensor("dbg_h1", [npc, HC], BF16, kind="ExternalOutput")
        dbg_xf = nc.dram_tensor("dbg_xf", [NCORES * npc, XW], BF16, kind="ExternalOutput")

    with tile.TileContext(nc) as tc:
        with (
            tc.tile_pool(name="dram", bufs=1, space="DRAM") as dram,
            tc.tile_pool(name="const", bufs=1) as cpool,
            tc.tile_pool(name="win", bufs=2) as wpool,
            tc.tile_pool(name="chunk", bufs=3) as kpool,
            tc.tile_pool(name="ps2", bufs=2, space="PSUM") as ps2,
            tc.tile_pool(name="ps1", bufs=1, space="PSUM") as ps1,
        ):
            h_full = dram.tile([npad, HID], BF16)
            h1loc = dram.tile([npc, HC], BF16)
            xl2loc = dram.tile([npc, XW], BF16)
            xl2full = dram.tile([NCORES * npc, XW], BF16)

            def cload(name, shape, dt, src_d):
                t = cpool.tile(shape, dt, tag=name)
                nc.sync.dma_start(t[:], src_d[:, :])
                return t

            ident = cload("ident", [P, P], BF16, ident_d)
            iota = cload("iota", [P, P], BF16, iota_d)
            ones1 = cload("ones1", [1, P], BF16, ones1_d)
            mlp1 = cload("mlp1", [F_IN + 1, HID], BF16, mlp1_d)
            mlp2 = cload("mlp2", [HID + 1, HID], BF16, mlp2_d)
            rhs1c = cload("rhs1c", [P, HC + H], BF16, rhs1c_d)
            wr1 = cload("wr1", [HID, HC + H], BF16, wr1_d)
            rhsxl1 = cload("rhsxl1", [P, HC], BF16, rhsxl1_d)
            rhs2c = cload("rhs2c", [P, HC + 1], BF16, rhs2c_d)
            wr2a = cload("wr2a", [P, HC + 1], BF16, wr2a_d)
            wr2b = cload("wr2b", [P, HC + 1], BF16, wr2b_d)
            wl2a = cload("wl2a", [P, HC + 1], BF16, wl2a_d)
            wl2b = cload("wl2b", [P, HC + 1], BF16, wl2b_d)
            xl2bias = cload("xl2bias", [1, HC + 1], BF16, xl2bias_d)
            att2b = cload("att2b", [P, HC], BF16, att2b_d)
            zeros = cpool.tile([P, HC + H], BF16)
            nc.vector.memset(zeros[:], 0.0)
            neg4 = cpool.tile([P, 1], F32)
            nc.vector.memset(neg4[:], -4.0)

            # ---------------- phase 0: MLP encoder -> h_full ----------------
            for i in range(nch0):
                sl = slice(i * 512, (i + 1) * 512)
                rx = kpool.tile([F_IN + 1, 512], BF16, tag="mlp_rx")
                nc.sync.dma_start(rx[:], xt_d[:, sl])
                p1 = ps1.tile([HID, 512], F32, tag="mlp_ps")
                nc.tensor.matmul(p1[:], lhsT=mlp1[:], rhs=rx[:], start=True, stop=True)
                ht = kpool.tile([HID + 1, 512], BF16, tag="mlp_ht")
                nc.scalar.activation(ht[0:HID, :], p1[:], AF.Relu)
                nc.vector.memset(ht[HID:HID + 1, :], 1.0)
                p2 = ps1.tile([HID, 512], F32, tag="mlp_ps")
                nc.tensor.matmul(p2[:], lhsT=mlp2[:], rhs=ht[:], start=True, stop=True)
                h2 = kpool.tile([HID, 512], BF16, tag="mlp_h2")
                nc.scalar.activation(h2[:], p2[:], AF.Relu)
                hrow = kpool.tile([P, 4, HID], BF16, tag="mlp_hrow")
                for j in range(4):
                    pt = ps1.tile([P, HID], BF16, tag="pst")
                    nc.tensor.transpose(pt[:], h2[:, j * P:(j + 1) * P], ident[0:HID, 0:HID])
                    nc.scalar.activation(hrow[:, j, :], pt[:], AF.Copy)
                nc.sync.dma_start(
                    h_full[sl, :].rearrange("(j p) d -> p j d", p=P), hrow[:])

            # ---------------- phase 1: GAT layer 1 --------------------------
            ab_sl1 = [(h * HID, (h + 1) * HID) for h in range(H)]
            for w in range(nw1):
                span = min(w1, npc - w * w1)
                nb = w * w1
                hw = wpool.tile([w1, HID], BF16, tag="hw")
                nc.sync.dma_start(hw[0:span, :], h_full[nb:nb + span, :])
                pt = ps1.tile([HID, w1], BF16, tag="pst")
                nc.tensor.transpose(pt[:, 0:span], hw[0:span, :], ident[0:span, 0:span])
                hwT = wpool.tile([HID, w1], BF16, tag="hwT")
                nc.scalar.activation(hwT[:, 0:span], pt[:, 0:span], AF.Copy)
                pxr = ps1.tile([w1, HC + H], F32, tag="ps_prep")
                nc.tensor.matmul(pxr[0:span, :], lhsT=hwT[:, 0:span], rhs=wr1[:],
                                 start=True, stop=True)
                rstk = wpool.tile([P, HC + H], BF16, tag="rstk")
                nc.scalar.activation(rstk[:], rhs1c[:], AF.Copy)
                xrw = wpool.tile([w1, HC + H], BF16, tag="xrw")
                nc.scalar.activation(xrw[0:span, :], pxr[0:span, :], AF.Copy)
                nc.sync.dma_start(rstk[72:72 + span, :], xrw[0:span, :])

                srcw = wpool.tile([P, k1], I32, tag="srcw")
                nc.sync.dma_start(srcw[:], src1_d[:, w * k1:(w + 1) * k1])
                dstw = wpool.tile([P, k1], F32, tag="dstw")
                nc.sync.dma_start(dstw[:], dstl1_d[:, w * k1:(w + 1) * k1])
                pre = wpool.tile([P, k1, P], BF16, tag="pre1")
                nc.sync.dma_start(
                    pre[:, :, HID:HID + ED],
                    ea1_d[:, w * k1 * ED:(w + 1) * k1 * ED].rearrange(
                        "p (k d) -> p k d", d=ED))
                nc.vector.memset(pre[:, :, 127:P], 1.0)
                for c in range(k1):
                    nc.vector.tensor_scalar(
                        out=pre[:, c, 72:127], in0=iota[:, 0:55],
                        scalar1=dstw[:, c:c + 1], scalar2=None, op0=ALU.is_equal)
                for c in range(k1):
                    nc.gpsimd.indirect_dma_start(
                        out=pre[:, c, 0:HID], out_offset=None,
                        in_=h_full[:, :],
                        in_offset=IndirectOffsetOnAxis(ap=srcw[:, c:c + 1], axis=0))

                pout = ps2.tile([P, HC + H], F32, tag="ps_out")
                for c in range(k1):
                    pt2 = ps1.tile([P, P], BF16, tag="pst")
                    nc.tensor.transpose(pt2[:], pre[:, c, :], ident[:])
                    stk = kpool.tile([P, P], BF16, tag="stk")
                    nc.scalar.activation(stk[:], pt2[:], AF.Copy)
                    pss = ps2.tile([P, HC + H], F32, tag="ps_s")
                    nc.tensor.matmul(pss[:], lhsT=stk[:], rhs=rstk[:], start=True, stop=True)
                    psx = ps1.tile([P, HC], F32, tag="ps_xl")
                    nc.tensor.matmul(psx[:], lhsT=stk[:], rhs=rhsxl1[:], start=True, stop=True)
                    abss = kpool.tile([P, HC], BF16, tag="abss")
                    nc.scalar.activation(abss[:], pss[:, 0:HC], AF.Abs)
                    scr = kpool.tile([P, HC], BF16, tag="scr")
                    aabs = kpool.tile([P, 2 * H], F32, tag="aabs")
                    for h in range(H):
                        a, b = h * HID, (h + 1) * HID
                        mid = a + npos1[h]
                        if mid > a:
                            nc.vector.tensor_scalar(
                                out=scr[:, a:mid], in0=abss[:, a:mid], scalar1=1.0,
                                scalar2=None, op0=ALU.mult, op1=ALU.add,
                                accum_out=aabs[:, h:h + 1])
                        else:
                            nc.vector.memset(aabs[:, h:h + 1], 0.0)
                        if b > mid:
                            nc.vector.tensor_scalar(
                                out=scr[:, mid:b], in0=abss[:, mid:b], scalar1=1.0,
                                scalar2=None, op0=ALU.mult, op1=ALU.add,
                                accum_out=aabs[:, H + h:H + h + 1])
                        else:
                            nc.vector.memset(aabs[:, H + h:H + h + 1], 0.0)
                    t1 = kpool.tile([P, H], F32, tag="t1")
                    nc.vector.scalar_tensor_tensor(
                        out=t1[:], in0=aabs[:, 0:H], scalar=0.4,
                        in1=pss[:, HC:HC + H], op0=ALU.mult, op1=ALU.add)
                    alpha = kpool.tile([P, H], F32, tag="alpha")
                    nc.vector.scalar_tensor_tensor(
                        out=alpha[:], in0=aabs[:, H:2 * H], scalar=-0.4,
                        in1=t1[:], op0=ALU.mult, op1=ALU.add)
                    ex = kpool.tile([P, H], F32, tag="ex")
                    nc.scalar.activation(ex[:], alpha[:], AF.Exp, bias=neg4[:])
                    pay = kpool.tile([P, HC + H], BF16, tag="pay")
                    for h, (a, b) in enumerate(ab_sl1):
                        if h < 2:
                            nc.scalar.activation(pay[:, a:b], psx[:, a:b], AF.Copy,
                                                 scale=ex[:, h:h + 1])
                        else:
                            nc.vector.tensor_scalar(
                                out=pay[:, a:b], in0=psx[:, a:b],
                                scalar1=ex[:, h:h + 1], scalar2=None, op0=ALU.mult)
                    nc.vector.tensor_copy(pay[:, HC:HC + H], ex[:])
                    nc.tensor.matmul(pout[0:span, :], lhsT=pre[:, c, 72:72 + span],
                                     rhs=pay[:], start=(c == 0), stop=(c == k1 - 1))

                deng = wpool.tile([w1, H], F32, tag="deng")
                nc.vector.tensor_scalar(out=deng[0:span, :], in0=pout[0:span, HC:HC + H],
                                        scalar1=1e-30, scalar2=None, op0=ALU.max)
                rden = wpool.tile([w1, H], F32, tag="rden")
                nc.vector.reciprocal(rden[0:span, :], deng[0:span, :])
                h1w = wpool.tile([w1, HC], BF16, tag="h1w")
                for h, (a, b) in enumerate(ab_sl1):
                    nc.vector.tensor_scalar(
                        out=h1w[0:span, a:b], in0=pout[0:span, a:b],
                        scalar1=rden[0:span, h:h + 1], scalar2=0.0,
                        op0=ALU.mult, op1=ALU.max)
                nc.sync.dma_start(h1loc[nb:nb + span, :], h1w[0:span, :])

                pxt = ps1.tile([P, w1], BF16, tag="pst")
                h1T0 = wpool.tile([P, w1], BF16, tag="h1T0")
                nc.tensor.transpose(pxt[:, 0:span], h1w[0:span, 0:P], ident[0:span, 0:span])
                nc.scalar.activation(h1T0[:, 0:span], pxt[:, 0:span], AF.Copy)
                pxt2 = ps1.tile([P, w1], BF16, tag="pst")
                h1T1 = wpool.tile([P, w1], BF16, tag="h1T1")
                nc.tensor.transpose(pxt2[:, 0:span], h1w[0:span, P:HC], ident[0:span, 0:span])
                nc.scalar.activation(h1T1[:, 0:span], pxt2[:, 0:span], AF.Copy)
                pxl2 = ps1.tile([w1, HC + 1], F32, tag="ps_prep")
                nc.tensor.matmul(pxl2[0:span, :], lhsT=h1T0[:, 0:span], rhs=wl2a[:],
                                 start=True, stop=False)
                nc.tensor.matmul(pxl2[0:span, :], lhsT=h1T1[:, 0:span], rhs=wl2b[:],
                                 start=False, stop=False)
                nc.tensor.matmul(pxl2[0:span, :], lhsT=ones1[:, 0:span], rhs=xl2bias[:],
                                 start=False, stop=True)
                xl2w = wpool.tile([w1, XW], BF16, tag="xl2w")
                nc.scalar.activation(xl2w[0:span, 0:HC + 1], pxl2[0:span, :], AF.Copy)
                # lo residual of the t_lin column (bf16 hi/lo pair)
                tl_lo = wpool.tile([w1, 1], BF16, tag="tl_lo")
                nc.vector.tensor_tensor(
                    out=tl_lo[0:span, :], in0=pxl2[0:span, HC:HC + 1],
                    in1=xl2w[0:span, HC:HC + 1], op=ALU.subtract)
                nc.vector.tensor_copy(xl2w[0:span, HC + 1:XW], tl_lo[0:span, :])
                nc.sync.dma_start(xl2loc[nb:nb + span, :], xl2w[0:span, :])

            # ---------------- phase 2: allgather xl2 table -------------------
            nc.gpsimd.collective_compute(
                "AllGather", ALU.bypass,
                replica_groups=[list(range(NCORES))],
                ins=[xl2loc[:]], outs=[xl2full[:]])

            # ---------------- phase 3: GAT layer 2 --------------------------
            for w in range(nw2):
                span = min(w2, npc - w * w2)
                nb = w * w2
                h1r = wpool.tile([w2, HC], BF16, tag="h1r")
                nc.sync.dma_start(h1r[0:span, :], h1loc[nb:nb + span, :])
                pt0 = ps1.tile([P, w2], BF16, tag="pst")
                hrT0 = wpool.tile([P, w2], BF16, tag="hrT0")
                nc.tensor.transpose(pt0[:, 0:span], h1r[0:span, 0:P], ident[0:span, 0:span])
                nc.scalar.activation(hrT0[:, 0:span], pt0[:, 0:span], AF.Copy)
                pt1 = ps1.tile([P, w2], BF16, tag="pst")
                hrT1 = wpool.tile([P, w2], BF16, tag="hrT1")
                nc.tensor.transpose(pt1[:, 0:span], h1r[0:span, P:HC], ident[0:span, 0:span])
                nc.scalar.activation(hrT1[:, 0:span], pt1[:, 0:span], AF.Copy)
                pxr2 = ps1.tile([w2, HC + 1], F32, tag="ps_prep")
                nc.tensor.matmul(pxr2[0:span, :], lhsT=hrT0[:, 0:span], rhs=wr2a[:],
                                 start=True, stop=False)
                nc.tensor.matmul(pxr2[0:span, :], lhsT=hrT1[:, 0:span], rhs=wr2b[:],
                                 start=False, stop=True)
                rstk2 = wpool.tile([P, HC + 1], BF16, tag="rstk2")
                nc.scalar.activation(rstk2[:], rhs2c[:], AF.Copy)
                xrw2 = wpool.tile([w2, HC + 1], BF16, tag="xrw2")
                nc.scalar.activation(xrw2[0:span, :], pxr2[0:span, :], AF.Copy)
                nc.sync.dma_start(rstk2[ED:ED + span, :], xrw2[0:span, :])

                srcw2 = wpool.tile([P, k2], I32, tag="srcw2")
                nc.sync.dma_start(srcw2[:], src2_d[:, w * k2:(w + 1) * k2])
                dstw2 = wpool.tile([P, k2], F32, tag="dstw2")
                nc.sync.dma_start(dstw2[:], dstl2_d[:, w * k2:(w + 1) * k2])
                pre2 = wpool.tile([P, k2, P], BF16, tag="pre2")
                nc.sync.dma_start(
                    pre2[:, :, 0:ED],
                    ea2_d[:, w * k2 * ED:(w + 1) * k2 * ED].rearrange(
                        "p (k d) -> p k d", d=ED))
                nc.vector.memset(pre2[:, :, 127:P], 1.0)
                for c in range(k2):
                    nc.vector.tensor_scalar(
                        out=pre2[:, c, ED:ED + 119], in0=iota[:, 0:119],
                        scalar1=dstw2[:, c:c + 1], scalar2=None, op0=ALU.is_equal)
                xg = wpool.tile([P, k2, XW], BF16, tag="xg")
                for c in range(k2):
                    nc.gpsimd.indirect_dma_start(
                        out=xg[:, c, :], out_offset=None,
                        in_=xl2full[:, :],
                        in_offset=IndirectOffsetOnAxis(ap=srcw2[:, c:c + 1], axis=0))

                pout2 = ps2.tile([P, HC + 1], F32, tag="ps_out")
                for c in range(k2):
                    pt2 = ps1.tile([P, P], BF16, tag="pst")
                    nc.tensor.transpose(pt2[:], pre2[:, c, :], ident[:])
                    stk2 = kpool.tile([P, P], BF16, tag="stk")
                    nc.scalar.activation(stk2[:], pt2[:], AF.Copy)
                    pss2 = ps2.tile([P, HC + 1], F32, tag="ps_s")
                    nc.tensor.matmul(pss2[:], lhsT=stk2[:], rhs=rstk2[:],
                                     start=True, stop=False)
                    nc.tensor.matmul(pss2[:, 0:P], lhsT=ident[:], rhs=xg[:, c, 0:P],
                                     start=False, stop=False)
                    nc.tensor.matmul(pss2[:, P:HC], lhsT=ident[:], rhs=xg[:, c, P:HC],
                                     start=False, stop=True)
                    scr2 = kpool.tile([P, HC], BF16, tag="abss")
                    nc.scalar.activation(scr2[:], pss2[:, 0:HC], AF.Abs)
                    wabs2 = kpool.tile([P, HC], BF16, tag="scr")
                    aabs2 = kpool.tile([P, 1], F32, tag="aabs")
                    nc.vector.scalar_tensor_tensor(
                        out=wabs2[:], in0=scr2[:], scalar=1.0,
                        in1=att2b[:], op0=ALU.mult, op1=ALU.mult,
                        accum_out=aabs2[:])
                    alpha2 = kpool.tile([P, 1], F32, tag="alpha")
                    nc.vector.tensor_tensor(
                        out=alpha2[:], in0=aabs2[:], in1=pss2[:, HC:HC + 1],
                        op=ALU.add)
                    tlin = kpool.tile([P, 1], F32, tag="t1")
                    nc.vector.tensor_tensor(
                        out=tlin[:], in0=xg[:, c, HC:HC + 1],
                        in1=xg[:, c, HC + 1:XW], op=ALU.add)
                    ex2 = kpool.tile([P, 1], F32, tag="ex")
                    nc.scalar.activation(ex2[:], alpha2[:], AF.Exp,
                                         bias=tlin[:], scale=1.0)
                    pay2 = kpool.tile([P, HC + 1], BF16, tag="pay")
                    nc.scalar.activation(pay2[:, 0:P], xg[:, c, 0:P], AF.Copy,
                                         scale=ex2[:])
                    nc.vector.tensor_scalar(
                        out=pay2[:, P:HC], in0=xg[:, c, P:HC],
                        scalar1=ex2[:], scalar2=None, op0=ALU.mult)
                    nc.vector.tensor_copy(pay2[:, HC:HC + 1], ex2[:])
                    nc.tensor.matmul(pout2[0:span, :], lhsT=pre2[:, c, ED:ED + span],
                                     rhs=pay2[:], start=(c == 0), stop=(c == k2 - 1))

                deng2 = wpool.tile([w2, 1], F32, tag="deng")
                nc.vector.tensor_scalar(out=deng2[0:span, :], in0=pout2[0:span, HC:HC + 1],
                                        scalar1=1e-30, scalar2=None, op0=ALU.max)
                rden2 = wpool.tile([w2, 1], F32, tag="rden")
                nc.vector.reciprocal(rden2[0:span, :], deng2[0:span, :])
                outw = wpool.tile([w2, HC], F32, tag="outw")
                nc.vector.tensor_scalar(
                    out=outw[0:span, :], in0=pout2[0:span, 0:HC],
                    scalar1=rden2[0:span, :], scalar2=0.0, op0=ALU.mult, op1=ALU.max)
                nc.sync.dma_start(out_d[nb:nb + span, :], outw[0:span, :])

            if debug:
                nc.sync.dma_start(dbg_h[:, :], h_full[:, :])
                nc.sync.dma_start(dbg_h1[:, :], h1loc[:, :])
                nc.sync.dma_start(dbg_xf[:, :], xl2full[:, :])

    nc.finalize()
    return nc


# ----------------------------------------------------------------------------
# entry point
# ----------------------------------------------------------------------------

def _install_ntff_hook():
    """Shim antenv.axon_hooks so trace=True can collect NTFF profiles."""
    import types
    try:
        from antenv.axon_hooks import get_axon_ntff_profile_hook  # noqa: F401
        return
    except ImportError:
        pass
    try:
        import antenv
        boot_dir = "/root/.axon_site/trn_agent_boot"
        so_path = "/opt/axon/libaxon_pjrt.so"
        if boot_dir not in sys.path:
            sys.path.insert(0, boot_dir)
        import trn_boot
        mod = types.ModuleType("antenv.axon_hooks")
        _state = {"hook": None}
        mod.set_axon_ntff_profile_hook = lambda h: _state.__setitem__("hook", h)
        mod.get_axon_ntff_profile_hook = lambda: _state["hook"]
        sys.modules["antenv.axon_hooks"] = mod
        antenv.axon_hooks = mod
        if os.path.exists(so_path):
            mod.set_axon_ntff_profile_hook(
                trn_boot._ntff_profile_via_ctypes(so_path))
    except Exception as exc:  # profiling is best-effort
        print("ntff hook install failed:", exc)


def run(inputs, trace=False):
    if trace:
        _install_ntff_hook()
    n = int(inputs["x"].shape[0])
    e = int(inputs["edge_index"].shape[1])
    assert n % NCORES == 0
    npc = n // NCORES
    meta, in_maps, perms = _prep_host(inputs, n, e, npc, w1=55, w2=119)
    nc = _build_nc(meta)
    res = run_bass_kernel_spmd(nc, in_maps, list(range(NCORES)), trace=trace)
    full = np.empty((n, HC), np.float32)
    for c in range(NCORES):
        full[c * npc + perms[c]] = res.results[c]["out"]
    return full, res


def kernel(**inputs):
    full, _ = run(inputs, trace=False)
    return full
